# revision 19
# baseline (speedup 1.0000x reference)
"""Trainium2 Bass kernel for ComputeAllAtomCoords.

Strategy (see problem spec: B=32, L=4096, 22 aa types, 27 atoms, 8 cores):
  - Shard batch across 8 cores (4 batch rows / core, T=16384 tokens each).
  - Host sorts each core's tokens by aa type (seq value). All per-type
    tables (RT matrices, rot-axis consts, base coords, frame-selection
    indices, atom masks) become piecewise-constant along the sorted
    stream. Tokens are laid out [128 partitions x W columns] with every
    partition holding tokens of a single type, so per-type constants are
    [P,1] per-partition scalars -> fused 1-op MACs (scalar_tensor_tensor)
    on the Vector/GpSimd engines and scale/bias operands on Scalar engine.
  - The full 4x4-matrix frame chain (RTF0..RTF8) is computed SoA style
    (tokens across partitions+free dim) with merged 4-row strided APs.
  - The final "gather frame by base_indices and apply to base coords"
    einsum collapses, per type, into a fixed [108 -> 81] linear map of the
    token's flattened frames. Tokens of one type occupy a contiguous
    partition range, so after a PE transpose (token-major -> feature-major)
    it becomes a plain matmul per type with the per-type matrix baked on
    the host. fp32r at N>=256 runs at full PE rate.
  - Atom masking (NaN fill) is applied host-side during un-permutation.
"""

import hashlib
import sys

import numpy as np

sys.path.insert(0, "/opt/trn_rl_repo")

import concourse.bass as bass  # noqa: E402
import concourse.bacc as bacc  # noqa: E402
import concourse.mybir as mybir  # noqa: E402
import concourse.tile as tile  # noqa: E402

F32 = mybir.dt.float32
F32R = mybir.dt.float32r

# Problem shapes (hardcoded per contest rules)
B, L, NAA, NAT = 32, 4096, 22, 27
N_CORES = 8
BPC = B // N_CORES          # batch rows per core
T = BPC * L                 # tokens per core

# ptab column map
PT_RT = 0                   # 7*16 RT consts: PT_RT + kk*16 + m*4 + j
PT_A1 = 112                 # rot-axis1 ct-coeff 3x3 (i*3+j)
PT_B1 = 121                 # rot-axis1 st-coeff
PT_C1 = 130                 # rot-axis1 const
PT_A2 = 139
PT_B2 = 148
PT_C2 = 157
PT_N = 166


# ----------------------------------------------------------------------------
# host-side tables (mirror reference.py math in float32)
# ----------------------------------------------------------------------------

def make_tables(base_indices, RTs, xyzs_base, mask):
    """Per-type constants. All float32, mirroring reference.py ops."""
    bi = np.asarray(base_indices).astype(np.int64)          # [22,27]
    rt = np.asarray(RTs, np.float32)                        # [22,7,4,4]
    bx = np.asarray(xyzs_base, np.float32)                  # [22,27,4]
    mk = np.asarray(mask).astype(bool)                      # [22,27]

    # ax1/ax2 per type (functions of xyzs_base only) -- float32 math
    NCr = (0.5 * (bx[:, 2, :3] + bx[:, 0, :3])).astype(np.float32)
    CAr = bx[:, 1, :3]
    CBr = bx[:, 4, :3]
    ax1 = np.cross(CBr - CAr, NCr - CAr).astype(np.float32)
    ax1 = ax1 / (np.sqrt((ax1 * ax1).sum(-1, keepdims=True)) + np.float32(1e-8))
    NCp = bx[:, 2, :3] - bx[:, 0, :3]
    NCpp = NCp - ((NCp * NCr).sum(-1, keepdims=True)
                  / (NCr * NCr).sum(-1, keepdims=True)) * NCr
    ax2 = np.cross(CBr - CAr, NCpp).astype(np.float32)
    ax2 = ax2 / (np.sqrt((ax2 * ax2).sum(-1, keepdims=True)) + np.float32(1e-8))

    def axis_coeffs(u):
        # rot_axis(a, u) = ct*A + st*Bm + C  (3x3 blocks)
        uu = np.einsum("si,sj->sij", u, u).astype(np.float32)   # [22,3,3]
        A = np.eye(3, dtype=np.float32)[None] - uu
        C = uu
        Bm = np.zeros((NAA, 3, 3), np.float32)
        Bm[:, 0, 1] = -u[:, 2]; Bm[:, 0, 2] = u[:, 1]
        Bm[:, 1, 0] = u[:, 2];  Bm[:, 1, 2] = -u[:, 0]
        Bm[:, 2, 0] = -u[:, 1]; Bm[:, 2, 1] = u[:, 0]
        return A, Bm, C

    A1, B1, C1 = axis_coeffs(ax1.astype(np.float32))
    A2, B2, C2 = axis_coeffs(ax2.astype(np.float32))

    # einsum matrices: LT[s] maps 108 frame feats -> 81 coords
    # feat r = k*12 + i*4 + j (rows i<3 of each frame k, in the transposed
    # rhs built from rtfs feats [[16,9],[1,12]]); out m = a*3 + i
    LT = np.zeros((NAA, 108, 81), np.float32)
    for s in range(NAA):
        for a in range(NAT):
            k = int(bi[s, a])
            for i in range(3):
                for j in range(4):
                    LT[s, k * 12 + i * 4 + j, a * 3 + i] = bx[s, a, j]

    return dict(rt=rt, A1=A1, B1=B1, C1=C1, A2=A2, B2=B2, C2=C2,
                LT=LT, mask=mk, bi=bi)


# ----------------------------------------------------------------------------
# layout planning (shared across cores -> single SPMD program)
# ----------------------------------------------------------------------------

def make_plan(seq):
    """Partition/column layout shared by all 8 cores."""
    seq = np.asarray(seq).astype(np.int64)
    counts = np.zeros((N_CORES, NAA), np.int64)
    for c in range(N_CORES):
        sc = seq[c * BPC:(c + 1) * BPC].reshape(-1)
        counts[c] = np.bincount(sc, minlength=NAA)
    cmax = counts.max(0)                      # worst-case bucket per type

    W = 136
    while True:
        n_s = np.maximum((cmax + W - 1) // W, (cmax > 0).astype(np.int64))
        if n_s.sum() <= 128:
            break
        W += 8
    n_s = n_s.astype(np.int64)
    p_start = np.concatenate([[0], np.cumsum(n_s)])[:NAA]
    P_used = int(n_s.sum())

    # per-partition type id (unused partitions -> type 0)
    ptype = np.zeros(128, np.int64)
    for s in range(NAA):
        ptype[p_start[s]:p_start[s] + n_s[s]] = s

    return dict(W=int(W), n_s=n_s, p_start=p_start, P_used=P_used,
                ptype=ptype, counts=counts)


def core_slots(plan, seq_core):
    """tokmap [128, W] (token index per slot) + valid mask."""
    W = plan["W"]
    tokmap = np.zeros((128, W), np.int64)
    valid = np.zeros((128, W), bool)
    order = np.argsort(seq_core, kind="stable")
    cnt = np.bincount(seq_core, minlength=NAA)
    off = np.concatenate([[0], np.cumsum(cnt)])
    for s in range(NAA):
        toks = order[off[s]:off[s + 1]]
        ns = int(plan["n_s"][s])
        p0 = int(plan["p_start"][s])
        nslot = ns * W
        if len(toks) == 0:
            continue
        pad = np.full(nslot, toks[0], np.int64)
        pad[:len(toks)] = toks
        tokmap[p0:p0 + ns] = pad.reshape(ns, W)
        v = np.zeros(nslot, bool); v[:len(toks)] = True
        valid[p0:p0 + ns] = v.reshape(ns, W)
    return tokmap, valid


def build_ptab(plan, tables):
    W = plan["W"]; del W
    pt = np.zeros((128, PT_N), np.float32)
    tp = plan["ptype"]
    rt = tables["rt"]
    pt[:, PT_RT:PT_RT + 112] = rt[tp].reshape(128, 112)
    for base, key in ((PT_A1, "A1"), (PT_B1, "B1"), (PT_C1, "C1"),
                      (PT_A2, "A2"), (PT_B2, "B2"), (PT_C2, "C2")):
        pt[:, base:base + 9] = tables[key][tp].reshape(128, 9)
    return pt


def pack_inputs(plan, tokmap, xyz_core, alphas_core):
    """xa [128, 29*W] feature-major: feats 0..8 = N,Ca,C xyz; 9..18 = alpha
    cos comps (pairs 0..9); 19..28 = alpha sin comps."""
    W = plan["W"]
    x9 = xyz_core.reshape(T, 9)
    al = alphas_core.reshape(T, 10, 2)
    xa = np.zeros((128, 29, W), np.float32)
    tm = tokmap
    xa[:, 0:9, :] = x9[tm].transpose(0, 2, 1)
    xa[:, 9:19, :] = al[tm][:, :, :, 0].transpose(0, 2, 1)
    xa[:, 19:29, :] = al[tm][:, :, :, 1].transpose(0, 2, 1)
    return xa.reshape(128, 29 * W)


# ----------------------------------------------------------------------------
# numpy mirror of the device computation (layout-level validation)
# ----------------------------------------------------------------------------

def numpy_mirror(plan, tables, xa):
    """Compute rtfs [128, W, 9, 16] and xyz3s [81, 128*W] like the device."""
    W = plan["W"]
    f = np.float32
    xa = xa.reshape(128, 29, W).astype(f)
    tp = plan["ptype"]
    rt = tables["rt"][tp]          # [128,7,4,4]
    N_, Ca, C_ = xa[:, 0:3], xa[:, 3:6], xa[:, 6:9]
    cq = xa[:, 9:19]; sq = xa[:, 19:29]

    # alpha normalization
    n = np.sqrt(cq * cq + sq * sq).astype(f) + f(1e-6)
    r = (f(1.0) / n).astype(f)
    ct, st = (cq * r).astype(f), (sq * r).astype(f)

    # frame 0
    v1 = C_ - Ca; v2 = N_ - Ca
    r1 = f(1.0) / (np.sqrt((v1 * v1).sum(1)).astype(f) + f(1e-8))
    e1 = v1 * r1[:, None]
    d2 = (e1 * v2).sum(1)
    u2 = v2 - e1 * d2[:, None]
    r2 = f(1.0) / (np.sqrt((u2 * u2).sum(1)).astype(f) + f(1e-8))
    e2 = u2 * r2[:, None]
    e3 = np.cross(e1, e2, axis=1).astype(f)

    RTF = np.zeros((128, W, 9, 4, 4), f)
    R0 = np.stack([e1, e2, e3], axis=2)          # [128,3,3,W] cols e1,e2,e3
    RTF[:, :, 0, :3, :3] = R0.transpose(0, 3, 1, 2)
    RTF[:, :, 0, :3, 3] = Ca.transpose(0, 2, 1)
    RTF[:, :, 0, 3, 3] = 1.0

    def rotX(q):
        m = np.zeros((128, W, 4, 4), f)
        m[..., 0, 0] = 1.0; m[..., 3, 3] = 1.0
        m[..., 1, 1] = ct[:, q]; m[..., 1, 2] = -st[:, q]
        m[..., 2, 1] = st[:, q]; m[..., 2, 2] = ct[:, q]
        return m

    def rotZ(q):
        m = np.zeros((128, W, 4, 4), f)
        m[..., 2, 2] = 1.0; m[..., 3, 3] = 1.0
        m[..., 0, 0] = ct[:, q]; m[..., 0, 1] = -st[:, q]
        m[..., 1, 0] = st[:, q]; m[..., 1, 1] = ct[:, q]
        return m

    def rax(q, An, Bn, Cn):
        A = tables[An][tp]; Bm = tables[Bn][tp]; Cc = tables[Cn][tp]
        m = np.zeros((128, W, 4, 4), f)
        m[..., :3, :3] = (ct[:, q, :, None, None] * A[:, None]
                          + st[:, q, :, None, None] * Bm[:, None]
                          + Cc[:, None]).astype(f)
        m[..., 3, 3] = 1.0
        return m

    mm = lambda a, b: np.matmul(a, b).astype(f)
    rtb = np.broadcast_to(rt[:, None], (128, W, 7, 4, 4))
    RTF[:, :, 1] = mm(mm(RTF[:, :, 0], rtb[:, :, 0]), rotX(0))
    RTF[:, :, 2] = mm(mm(RTF[:, :, 0], rtb[:, :, 1]), rotX(1))
    RTF[:, :, 3] = mm(mm(RTF[:, :, 0], rtb[:, :, 2]), rotX(2))
    RTF[:, :, 8] = mm(mm(RTF[:, :, 0], rax(7, "A1", "B1", "C1")),
                      rax(8, "A2", "B2", "C2"))
    RTF[:, :, 4] = mm(mm(mm(RTF[:, :, 8], rtb[:, :, 3]), rotX(3)), rotZ(9))
    RTF[:, :, 5] = mm(mm(RTF[:, :, 4], rtb[:, :, 4]), rotX(4))
    RTF[:, :, 6] = mm(mm(RTF[:, :, 5], rtb[:, :, 5]), rotX(5))
    RTF[:, :, 7] = mm(mm(RTF[:, :, 6], rtb[:, :, 6]), rotX(6))

    # einsum via per-type LT
    feats = RTF[:, :, :, :3, :].reshape(128, W, 108)     # k*12 + i*4 + j
    LTp = tables["LT"][tp]                               # [128,108,81]
    out = np.matmul(feats, LTp).transpose(2, 0, 1).astype(f)  # [81,128,W]
    return RTF, out.reshape(81, 128 * W)


def assemble_outputs(plan, tables, seq, fr_cores, r3_cores, xyz3s_cores,
                     tokmaps, valids):
    """fr_cores[c]: [9][P, W*12] rows0-2; r3_cores[c]: [P, W*36] rows3."""
    W = plan["W"]; P = plan["P_used"]
    RTframes = np.zeros((B, L, 9, 4, 4), np.float32)
    xyz3 = np.zeros((B, L, NAT, 3), np.float32)
    for c in range(N_CORES):
        vd = valids[c][:P].reshape(-1)
        toks = tokmaps[c][:P].reshape(-1)[vd]
        fr = np.stack([a.reshape(P, W, 3, 4) for a in fr_cores[c]], axis=2)
        fr = fr.reshape(P * W, 9, 3, 4)[vd]
        r3 = r3_cores[c].reshape(P, W, 9, 4).reshape(P * W, 9, 4)[vd]
        dst = RTframes[c * BPC:(c + 1) * BPC].reshape(T, 9, 4, 4)
        dst[toks, :, :3, :] = fr
        dst[toks, :, 3, :] = r3
        x = xyz3s_cores[c].reshape(81, 128, W)[:, :P].reshape(81, P * W)
        x = x.T[vd].reshape(-1, NAT, 3)
        xyz3[c * BPC:(c + 1) * BPC].reshape(T, NAT, 3)[toks] = x
    present = tables["mask"][np.asarray(seq).astype(np.int64)]   # [B,L,27]
    xyz3 = np.where(present[..., None], xyz3, np.float32(np.nan))
    return RTframes, xyz3


# ----------------------------------------------------------------------------
# bass program
# ----------------------------------------------------------------------------

def fap(base_ap, off, dims, parts=None):
    """AP with the tile's partition dim + custom free dims ([step,count])."""
    p = list(base_ap.ap[0])
    if parts is not None:
        p = [p[0], parts]
    return bass.AP(base_ap.tensor, base_ap.offset + off,
                   [p] + [list(d) for d in dims])


class Ledger:
    """Greedy engine load balancer for elementwise ops."""

    def __init__(self, nc):
        self.nc = nc
        self.load = {"v": 0.0, "a": 0.0, "g": 0.0}
        self.rate = {"v": 1.05, "a": 0.84, "g": 1.4}
        self.fixed = {"v": 135.0, "a": 225.0, "g": 135.0}

    def pick(self, allowed, fd, g_rate=None):
        best, bc = None, None
        for e in allowed:
            r = self.rate[e] if (e != "g" or g_rate is None) else g_rate
            c = self.load[e] + self.fixed[e] + r * fd
            if bc is None or c < bc:
                best, bc = e, c
        r = self.rate[best] if (best != "g" or g_rate is None) else g_rate
        self.load[best] += self.fixed[best] + r * fd
        return best

    def eng(self, e):
        return {"v": self.nc.vector, "a": self.nc.scalar, "g": self.nc.gpsimd}[e]


def build_program(plan):
    W = plan["W"]
    P = plan["P_used"]
    n_s = plan["n_s"]; p_start = plan["p_start"]
    Wc = 48                                    # einsum column chunk
    AL = mybir.AluOpType

    nc = bacc.Bacc("TRN2", target_bir_lowering=False, debug=False,
                   num_devices=N_CORES)

    d_xa = nc.dram_tensor("xa", [128, 29 * W], F32, kind="ExternalInput").ap()
    d_pt = nc.dram_tensor("ptab", [128, PT_N], F32, kind="ExternalInput").ap()
    d_lt = nc.dram_tensor("lt", [108, NAA * 81], F32, kind="ExternalInput").ap()
    d_id = nc.dram_tensor("ident", [128, 128], F32, kind="ExternalInput").ap()
    d_fr = [nc.dram_tensor(f"fr{k}", [P, W * 12], F32, kind="ExternalOutput").ap()
            for k in range(9)]
    d_r3 = nc.dram_tensor("row3", [P, W * 36], F32, kind="ExternalOutput").ap()
    d_x3 = nc.dram_tensor("xyz3s", [81, 128 * W], F32, kind="ExternalOutput").ap()

    with tile.TileContext(nc) as tc:
        import contextlib
        ctx = contextlib.ExitStack()
        with ctx:
            pool = ctx.enter_context(tc.tile_pool(name="main", bufs=1))
            ps_tr = ctx.enter_context(
                tc.tile_pool(name="ps_tr", bufs=2, space="PSUM"))
            ps_mm = ctx.enter_context(
                tc.tile_pool(name="ps_mm", bufs=2, space="PSUM"))
            xo_pool = ctx.enter_context(tc.tile_pool(name="xo", bufs=3))

            t_xa = pool.tile([128, 29 * W], F32, tag="xa")
            t_rtfs = pool.tile([128, 144 * W], F32, tag="rtfs")
            t_ta = pool.tile([128, 18 * W], F32, tag="ta")
            t_tb = pool.tile([128, 55 * W], F32, tag="tb")
            t_pt = pool.tile([128, PT_N], F32, tag="pt")
            t_lt = pool.tile([128, NAA * 81], F32, tag="lt")
            t_id = pool.tile([128, 128], F32, tag="ident")

            xa = t_xa[:]; rtfs = t_rtfs[:]; ta = t_ta[:]; tb = t_tb[:]
            pt = t_pt[:]; lt = t_lt[:]; idn = t_id[:]
            TR0 = 12 * W                      # TR region inside tb

            led = Ledger(nc)

            # ---------------- input DMAs ----------------
            nc.sync.dma_start(out=xa, in_=d_xa)
            nc.sync.dma_start(out=pt, in_=d_pt)
            nc.sync.dma_start(out=fap(lt, 0, [[1, NAA * 81]], parts=108),
                              in_=d_lt)
            nc.sync.dma_start(out=idn, in_=d_id)

            # AP helpers -----------------------------------------------------
            def xaf(f0, nf=1, bcast=None):
                if bcast is not None:
                    return fap(xa, f0 * W, [[0, bcast], [1, W]])
                return fap(xa, f0 * W, [[W, nf], [1, W]])

            def tbf(f0, nf=1, step=1):
                return fap(tb, f0 * W, [[W * step, nf], [1, W]])

            def tbb(f0, nf):
                return fap(tb, f0 * W, [[0, nf], [1, W]])

            def taf(f0, nf=1):
                return fap(ta, f0 * W, [[W, nf], [1, W]])

            def rf(k, i, j, ni=1):
                """rows0-2 feats: k*12 + i*4 + j, ni rows merged (step 4)."""
                return fap(rtfs, k * 12 + i * 4 + j, [[4, ni], [144, W]])

            def r3f(k, j, nj=1):
                """row3 feats: 108 + k*4 + j."""
                return fap(rtfs, 108 + k * 4 + j, [[1, nj], [144, W]])

            def pts(c):
                return fap(pt, c, [[1, 1]])

            def ptb(c, nf):
                return fap(pt, c, [[1, nf], [0, W]])

            def ew(e_allowed, emit, fd, g_rate=None):
                e = led.pick(e_allowed, fd, g_rate)
                emit(led.eng(e))

            def ts_mul(out, in0, scalar_ap, fd):
                e = led.pick(("v", "a"), fd)
                E = led.eng(e)
                if e == "a":
                    E.mul(out=out, in_=in0, mul=scalar_ap)
                else:
                    E.tensor_scalar(out=out, in0=in0, scalar1=scalar_ap,
                                    scalar2=None, op0=AL.mult)

            def const_write(out, c, fd):
                """out[:] = ptab const broadcast (out = 0*x + c)."""
                e = led.pick(("v", "a"), fd)
                E = led.eng(e)
                if e == "a":
                    E.activation(out=out, in_=xaf(0, 1),
                                 func=mybir.ActivationFunctionType.Identity,
                                 bias=pts(c), scale=0.0)
                else:
                    E.tensor_scalar(out=out, in0=xaf(0, 1), scalar1=0.0,
                                    scalar2=pts(c), op0=AL.mult, op1=AL.add)

            def stt(out, in0, c, fd, op1=AL.add):
                ew(("v",), lambda E: E.scalar_tensor_tensor(
                    out=out, in0=in0, scalar=pts(c), in1=out,
                    op0=AL.mult, op1=op1), fd)

            def tt(out, in0, in1, op, fd):
                ew(VG, lambda E: E.tensor_tensor(
                    out=out, in0=in0, in1=in1, op=op), fd)

            VG = ("v", "g")
            VAG = ("v", "a", "g")
            cA, sA = 9, 19

            def ctf(q, bcast=None):
                return xaf(cA + q, 1) if bcast is None else xaf(cA + q, bcast=bcast)

            def stf(q, bcast=None):
                return xaf(sA + q, 1) if bcast is None else xaf(sA + q, bcast=bcast)

            # ---------------- stage A: alpha normalization ------------------
            tt(tbf(0, 10), xaf(cA, 10), xaf(cA, 10), AL.mult, 10 * W)
            tt(tbf(10, 10), xaf(sA, 10), xaf(sA, 10), AL.mult, 10 * W)
            tt(tbf(0, 10), tbf(0, 10), tbf(10, 10), AL.add, 10 * W)
            nc.scalar.sqrt(out=tbf(10, 10), in_=tbf(0, 10))
            ew(VG, lambda E: E.tensor_scalar(
                out=tbf(10, 10), in0=tbf(10, 10), scalar1=1e-6, scalar2=None,
                op0=AL.add), 10 * W)
            nc.vector.reciprocal(out=tbf(0, 10), in_=tbf(10, 10))
            tt(xaf(cA, 10), xaf(cA, 10), tbf(0, 10), AL.mult, 10 * W)
            tt(xaf(sA, 10), xaf(sA, 10), tbf(0, 10), AL.mult, 10 * W)

            # ---------------- stage B: frame 0 ------------------------------
            tt(tbf(0, 3), xaf(6, 3), xaf(3, 3), AL.subtract, 3 * W)
            tt(tbf(3, 3), xaf(0, 3), xaf(3, 3), AL.subtract, 3 * W)
            tt(tbf(6, 3), tbf(0, 3), tbf(0, 3), AL.mult, 3 * W)
            nc.vector.tensor_reduce(
                out=tbf(9, 1), in_=fap(tb, 6 * W, [[1, W], [W, 3]]),
                axis=mybir.AxisListType.X, op=AL.add)
            nc.scalar.sqrt(out=tbf(10, 1), in_=tbf(9, 1))
            ew(VG, lambda E: E.tensor_scalar(
                out=tbf(10, 1), in0=tbf(10, 1), scalar1=1e-8, scalar2=None,
                op0=AL.add), W)
            nc.vector.reciprocal(out=tbf(9, 1), in_=tbf(10, 1))
            tt(rf(0, 0, 0, ni=3), tbf(0, 3), tbb(9, 3), AL.mult, 3 * W)
            tt(tbf(6, 3), rf(0, 0, 0, ni=3), tbf(3, 3), AL.mult, 3 * W)
            nc.vector.tensor_reduce(
                out=tbf(9, 1), in_=fap(tb, 6 * W, [[1, W], [W, 3]]),
                axis=mybir.AxisListType.X, op=AL.add)
            tt(tbf(6, 3), rf(0, 0, 0, ni=3), tbb(9, 3), AL.mult, 3 * W)
            tt(tbf(0, 3), tbf(3, 3), tbf(6, 3), AL.subtract, 3 * W)
            tt(tbf(6, 3), tbf(0, 3), tbf(0, 3), AL.mult, 3 * W)
            nc.vector.tensor_reduce(
                out=tbf(9, 1), in_=fap(tb, 6 * W, [[1, W], [W, 3]]),
                axis=mybir.AxisListType.X, op=AL.add)
            nc.scalar.sqrt(out=tbf(10, 1), in_=tbf(9, 1))
            ew(VG, lambda E: E.tensor_scalar(
                out=tbf(10, 1), in0=tbf(10, 1), scalar1=1e-8, scalar2=None,
                op0=AL.add), W)
            nc.vector.reciprocal(out=tbf(9, 1), in_=tbf(10, 1))
            tt(rf(0, 0, 1, ni=3), tbf(0, 3), tbb(9, 3), AL.mult, 3 * W)
            for cc in range(3):
                i1, i2 = (cc + 1) % 3, (cc + 2) % 3
                tt(tbf(6, 1), rf(0, i1, 0), rf(0, i2, 1), AL.mult, W)
                tt(tbf(7, 1), rf(0, i2, 0), rf(0, i1, 1), AL.mult, W)
                tt(rf(0, cc, 2), tbf(6, 1), tbf(7, 1), AL.subtract, W)
            nc.scalar.copy(out=rf(0, 0, 3, ni=3), in_=xaf(3, 3))
            nc.vector.memset(fap(rtfs, 108, [[1, 3], [144, W]]), 0.0)
            nc.vector.memset(fap(rtfs, 111, [[144, W]]), 1.0)
            # frame8 row3 = [0,0,0,1]
            nc.gpsimd.memset(fap(rtfs, 140, [[1, 3], [144, W]]), 0.0)
            nc.gpsimd.memset(fap(rtfs, 143, [[144, W]]), 1.0)

            # ---------------- stage C: rot-axis matrices --------------------
            for (q, ba, bb, bc, t0) in ((7, PT_A1, PT_B1, PT_C1, 0),
                                        (8, PT_A2, PT_B2, PT_C2, 9)):
                tt(taf(t0, 9), ctf(q, bcast=9), ptb(ba, 9), AL.mult, 9 * W)
                tt(tbf(12, 9), stf(q, bcast=9), ptb(bb, 9), AL.mult, 9 * W)
                tt(taf(t0, 9), taf(t0, 9), tbf(12, 9), AL.add, 9 * W)
                tt(taf(t0, 9), taf(t0, 9), ptb(bc, 9), AL.add, 9 * W)

            # ---------------- stage D: frame 8 rows0-2 ----------------------
            def p1f(j, ni=3):
                return fap(tb, (21 + j) * W, [[3 * W, ni], [1, W]])

            for (src_t0, dst) in ((0, "p1"), (9, "rtfs8")):
                for j in range(3):
                    for k in range(3):
                        if dst == "p1":
                            in0 = rf(0, 0, k, ni=3)
                            outp = p1f(j)
                        else:
                            in0 = p1f(k)
                            outp = rf(8, 0, j, ni=3)
                        a1 = fap(ta, (src_t0 + k * 3 + j) * W, [[0, 3], [1, W]])
                        if k == 0:
                            tt(outp, in0, a1, AL.mult, 3 * W)
                        else:
                            tt(tbf(30, 3), in0, a1, AL.mult, 3 * W)
                            tt(outp, outp, tbf(30, 3), AL.add, 3 * W)
            nc.scalar.copy(out=rf(8, 0, 3, ni=3), in_=rf(0, 0, 3, ni=3))

            # ------------- chain steps rows0-2 ------------------------------
            def chain_rows(frame, left, kk, q, g0, q0):
                def gcol(j):
                    return fap(tb, (g0 + (j - 1)) * W, [[2 * W, 3], [1, W]])

                for j in range(4):
                    outp = rf(frame, 0, j, ni=3) if j in (0, 3) else gcol(j)
                    for m in range(4):
                        c = PT_RT + kk * 16 + m * 4 + j
                        if m == 0:
                            ts_mul(outp, rf(left, 0, 0, ni=3), pts(c), 3 * W)
                        else:
                            stt(outp, rf(left, 0, m, ni=3), c, 3 * W)
                # rotX cols 1,2
                tt(rf(frame, 0, 1, ni=3), gcol(1), ctf(q, bcast=3), AL.mult, 3 * W)
                tt(tbf(q0, 3), gcol(2), stf(q, bcast=3), AL.mult, 3 * W)
                tt(rf(frame, 0, 1, ni=3), rf(frame, 0, 1, ni=3), tbf(q0, 3),
                   AL.add, 3 * W)
                tt(rf(frame, 0, 2, ni=3), gcol(2), ctf(q, bcast=3), AL.mult, 3 * W)
                tt(tbf(q0, 3), gcol(1), stf(q, bcast=3), AL.mult, 3 * W)
                tt(rf(frame, 0, 2, ni=3), rf(frame, 0, 2, ni=3), tbf(q0, 3),
                   AL.subtract, 3 * W)

            def chain_row3_const(frame, kk, q, tmp):
                """left row3 == [0,0,0,1]: G row3 = RT[kk] row3 (consts)."""
                c3 = lambda j: PT_RT + kk * 16 + 12 + j
                const_write(r3f(frame, 0), c3(0), W)
                const_write(r3f(frame, 3), c3(3), W)
                ts_mul(r3f(frame, 1), ctf(q), pts(c3(1)), W)
                stt(r3f(frame, 1), stf(q), c3(2), W)
                ts_mul(r3f(frame, 2), ctf(q), pts(c3(2)), W)
                ts_mul(tbf(tmp, 1), stf(q), pts(c3(1)), W)
                tt(r3f(frame, 2), r3f(frame, 2), tbf(tmp, 1), AL.subtract, W)

            def chain_row3_full(frame, left, kk, q, g0, tmp):
                """left row3 per-token: full G + rotX on row3."""
                c = lambda m, j: PT_RT + kk * 16 + m * 4 + j
                stage = {1: tbf(g0, 1), 2: tbf(g0 + 1, 1)}
                for j in range(4):
                    outp = r3f(frame, j) if j in (0, 3) else stage[j]
                    ts_mul(outp, r3f(left, 0), pts(c(0, j)), W)
                    for m in range(1, 4):
                        stt(outp, r3f(left, m), c(m, j), W)
                tt(r3f(frame, 1), stage[1], ctf(q), AL.mult, W)
                tt(tbf(tmp, 1), stage[2], stf(q), AL.mult, W)
                tt(r3f(frame, 1), r3f(frame, 1), tbf(tmp, 1), AL.add, W)
                tt(r3f(frame, 2), stage[2], ctf(q), AL.mult, W)
                tt(tbf(tmp, 1), stage[1], stf(q), AL.mult, W)
                tt(r3f(frame, 2), r3f(frame, 2), tbf(tmp, 1), AL.subtract, W)

            chain_rows(1, 0, 0, 0, g0=33, q0=39)
            chain_row3_const(1, 0, 0, tmp=42)
            chain_rows(2, 0, 1, 1, g0=43, q0=49)
            chain_row3_const(2, 1, 1, tmp=52)
            chain_rows(3, 0, 2, 2, g0=0, q0=6)
            chain_row3_const(3, 2, 2, tmp=9)

            # ---------------- frame 4 rows0-2 -------------------------------
            def g3col(j):
                return fap(tb, (21 + j) * W, [[3 * W, 3], [1, W]])

            for j in range(4):
                outp = rf(4, 0, j, ni=3) if j == 3 else g3col(j)
                for m in range(4):
                    c = PT_RT + 3 * 16 + m * 4 + j
                    if m == 0:
                        ts_mul(outp, rf(8, 0, 0, ni=3), pts(c), 3 * W)
                    else:
                        stt(outp, rf(8, 0, m, ni=3), c, 3 * W)
            # H1 = g1*ct3 + g2*st3 (tb 30-33); col2 = g2*ct3 - g1*st3
            tt(tbf(30, 3), g3col(1), ctf(3, bcast=3), AL.mult, 3 * W)
            tt(tbf(12, 3), g3col(2), stf(3, bcast=3), AL.mult, 3 * W)
            tt(tbf(30, 3), tbf(30, 3), tbf(12, 3), AL.add, 3 * W)
            tt(rf(4, 0, 2, ni=3), g3col(2), ctf(3, bcast=3), AL.mult, 3 * W)
            tt(tbf(12, 3), g3col(1), stf(3, bcast=3), AL.mult, 3 * W)
            tt(rf(4, 0, 2, ni=3), rf(4, 0, 2, ni=3), tbf(12, 3), AL.subtract,
               3 * W)
            # rotZ9: col0 = g0*c9 + H1*s9 ; col1 = H1*c9 - g0*s9
            tt(rf(4, 0, 0, ni=3), g3col(0), ctf(9, bcast=3), AL.mult, 3 * W)
            tt(tbf(12, 3), tbf(30, 3), stf(9, bcast=3), AL.mult, 3 * W)
            tt(rf(4, 0, 0, ni=3), rf(4, 0, 0, ni=3), tbf(12, 3), AL.add, 3 * W)
            tt(rf(4, 0, 1, ni=3), tbf(30, 3), ctf(9, bcast=3), AL.mult, 3 * W)
            tt(tbf(12, 3), g3col(0), stf(9, bcast=3), AL.mult, 3 * W)
            tt(rf(4, 0, 1, ni=3), rf(4, 0, 1, ni=3), tbf(12, 3), AL.subtract,
               3 * W)
            # frame 4 row3: rt3row3 @ rotX3 @ rotZ9 (rt3row3 const)
            c3 = lambda j: PT_RT + 3 * 16 + 12 + j
            ts_mul(tbf(15, 1), ctf(3), pts(c3(1)), W)       # a1
            stt(tbf(15, 1), stf(3), c3(2), W)
            ts_mul(r3f(4, 2), ctf(3), pts(c3(2)), W)        # a2 direct
            ts_mul(tbf(16, 1), stf(3), pts(c3(1)), W)
            tt(r3f(4, 2), r3f(4, 2), tbf(16, 1), AL.subtract, W)
            const_write(r3f(4, 3), c3(3), W)
            ts_mul(r3f(4, 0), ctf(9), pts(c3(0)), W)        # a0*c9
            tt(tbf(16, 1), tbf(15, 1), stf(9), AL.mult, W)  # a1*s9
            tt(r3f(4, 0), r3f(4, 0), tbf(16, 1), AL.add, W)
            tt(r3f(4, 1), tbf(15, 1), ctf(9), AL.mult, W)   # a1*c9
            ts_mul(tbf(16, 1), stf(9), pts(c3(0)), W)       # a0*s9
            tt(r3f(4, 1), r3f(4, 1), tbf(16, 1), AL.subtract, W)

            for (fr_i, left, kk, q) in ((5, 4, 4, 4), (6, 5, 5, 5), (7, 6, 6, 6)):
                chain_rows(fr_i, left, kk, q, g0=21, q0=27)
                chain_row3_full(fr_i, left, kk, q, g0=30, tmp=32)

            # ---------------- frame output DMAs -----------------------------
            for k in range(9):
                nc.sync.dma_start(
                    out=d_fr[k],
                    in_=fap(rtfs, k * 12, [[144, W], [1, 12]], parts=P))
            nc.sync.dma_start(
                out=d_r3, in_=fap(rtfs, 108, [[144, W], [1, 36]], parts=P))

            # ---------------- einsum ----------------------------------------
            col = 0
            while col < W:
                wch = min(Wc, W - col)
                ngrp = (wch + 3) // 4
                for g in range(ngrp):
                    pst = ps_tr.tile([128, 512], F32, tag="pstr")
                    nw = min(4, wch - g * 4)
                    for r in range(nw):
                        w = col + g * 4 + r
                        nc.tensor.matmul(
                            out=fap(pst[:], r * 128, [[1, 128]], parts=108),
                            lhsT=fap(rtfs, w * 144, [[1, 108]]),
                            rhs=idn, is_transpose=True,
                            start=True, stop=True)
                    # scatter-copy psum -> TR (tokens contiguous per type)
                    e = led.pick(("v", "a"), nw * 128)
                    cp_out = fap(tb, TR0 + g * 4, [[1, nw], [wch, 128]],
                                 parts=108)
                    cp_in = fap(pst[:], 0, [[128, nw], [1, 128]], parts=108)
                    if e == "a":
                        led.eng(e).copy(out=cp_out, in_=cp_in)
                    else:
                        led.eng(e).tensor_copy(out=cp_out, in_=cp_in)
                for s in range(NAA):
                    ns = int(n_s[s]); p0 = int(p_start[s])
                    if ns == 0:
                        continue
                    N = ns * wch
                    psm = ps_mm.tile([128, 512], F32, tag="psmm")
                    nc.tensor.matmul(
                        out=fap(psm[:], 0, [[1, N]], parts=81),
                        lhsT=fap(lt, s * 81, [[1, 81]], parts=108),
                        rhs=fap(tb, TR0 + p0 * wch, [[1, N]], parts=108),
                        start=True, stop=True)
                    xo = xo_pool.tile([128, 512], F32, tag="xo")
                    e = led.pick(("v", "a"), N)
                    xo_out = fap(xo[:], 0, [[1, N]], parts=81)
                    xo_in = fap(psm[:], 0, [[1, N]], parts=81)
                    if e == "a":
                        led.eng(e).copy(out=xo_out, in_=xo_in)
                    else:
                        led.eng(e).tensor_copy(out=xo_out, in_=xo_in)
                    nc.sync.dma_start(
                        out=bass.AP(d_x3.tensor, p0 * W + col,
                                    [[128 * W, 81], [W, ns], [1, wch]]),
                        in_=fap(xo[:], 0, [[wch, ns], [1, wch]], parts=81))
                col += wch

    nc.compile()
    return nc


# ----------------------------------------------------------------------------
# kernel entry
# ----------------------------------------------------------------------------

_CACHE = {}


def kernel(seq, xyz, alphas, base_indices, RTs_in_base_frame,
           xyzs_in_base_frame, allatom_mask):
    seq = np.asarray(seq).astype(np.int64)
    xyz = np.asarray(xyz, np.float32)
    alphas = np.asarray(alphas, np.float32)

    key = hashlib.sha1(
        seq.tobytes() + np.asarray(base_indices).astype(np.int64).tobytes()
        + np.asarray(RTs_in_base_frame, np.float32).tobytes()
        + np.asarray(xyzs_in_base_frame, np.float32).tobytes()
        + np.asarray(allatom_mask).astype(np.uint8).tobytes()).hexdigest()

    if key in _CACHE:
        plan, tables, nc, tokmaps, valids = _CACHE[key]
    else:
        tables = make_tables(base_indices, RTs_in_base_frame,
                             xyzs_in_base_frame, allatom_mask)
        plan = make_plan(seq)
        tokmaps, valids = [], []
        for c in range(N_CORES):
            tm, vd = core_slots(plan, seq[c * BPC:(c + 1) * BPC].reshape(-1))
            tokmaps.append(tm); valids.append(vd)
        nc = build_program(plan)
        _CACHE[key] = (plan, tables, nc, tokmaps, valids)

    ptab = build_ptab(plan, tables)
    ltp = np.ascontiguousarray(
        tables["LT"].transpose(1, 0, 2).reshape(108, NAA * 81))
    ident = np.eye(128, dtype=np.float32)

    in_maps = []
    for c in range(N_CORES):
        xa = pack_inputs(plan, tokmaps[c],
                         xyz[c * BPC:(c + 1) * BPC],
                         alphas[c * BPC:(c + 1) * BPC])
        in_maps.append({"xa": xa, "ptab": ptab, "lt": ltp, "ident": ident})

    from concourse.bass_utils import run_bass_kernel_spmd
    global _LAST
    _LAST = (nc, in_maps)
    res = run_bass_kernel_spmd(nc, in_maps, core_ids=list(range(N_CORES)))

    fr_cores = [[res.results[c][f"fr{k}"] for k in range(9)]
                for c in range(N_CORES)]
    r3_cores = [res.results[c]["row3"] for c in range(N_CORES)]
    xyz3s_cores = [res.results[c]["xyz3s"] for c in range(N_CORES)]
    return assemble_outputs(plan, tables, seq, fr_cores, r3_cores,
                            xyz3s_cores, tokmaps, valids)


_LAST = None


def _ensure_ntff_hook():
    """Shim antenv.axon_hooks if the image lacks it (boot degrades silently)."""
    import types
    try:
        from antenv.axon_hooks import get_axon_ntff_profile_hook  # noqa: F401
        return
    except ImportError:
        pass
    import antenv
    from trn_agent_boot.trn_boot import _ntff_profile_via_ctypes
    hook = [_ntff_profile_via_ctypes("/opt/axon/libaxon_pjrt.so")]
    mod = types.ModuleType("antenv.axon_hooks")
    mod.get_axon_ntff_profile_hook = lambda: hook[0]
    mod.set_axon_ntff_profile_hook = lambda h: hook.__setitem__(0, h)
    sys.modules["antenv.axon_hooks"] = mod
    antenv.axon_hooks = mod


def bench_hw(trace=True):
    """Re-run the last-compiled program with NTFF tracing for HW timing."""
    if _LAST is None:
        return None
    if trace:
        _ensure_ntff_hook()
    from concourse.bass_utils import run_bass_kernel_spmd
    nc, in_maps = _LAST
    return run_bass_kernel_spmd(nc, in_maps, core_ids=list(range(N_CORES)),
                                trace=trace)


# revision 21
# speedup vs baseline: 1.7692x; 1.7692x over previous
"""Trainium2 Bass kernel for ComputeAllAtomCoords.

Strategy (see problem spec: B=32, L=4096, 22 aa types, 27 atoms, 8 cores):
  - Shard batch across 8 cores (4 batch rows / core, T=16384 tokens each).
  - Host sorts each core's tokens by aa type (seq value). All per-type
    tables (RT matrices, rot-axis consts, base coords, frame-selection
    indices, atom masks) become piecewise-constant along the sorted
    stream. Tokens are laid out [128 partitions x W columns] with every
    partition holding tokens of a single type, so per-type constants are
    [P,1] per-partition scalars -> fused 1-op MACs (scalar_tensor_tensor)
    on the Vector/GpSimd engines and scale/bias operands on Scalar engine.
  - The full 4x4-matrix frame chain (RTF0..RTF8) is computed SoA style
    (tokens across partitions+free dim) with merged 4-row strided APs.
  - The final "gather frame by base_indices and apply to base coords"
    einsum collapses, per type, into a fixed [108 -> 81] linear map of the
    token's flattened frames. Tokens of one type occupy a contiguous
    partition range, so after a PE transpose (token-major -> feature-major)
    it becomes a plain matmul per type with the per-type matrix baked on
    the host. fp32r at N>=256 runs at full PE rate.
  - Atom masking (NaN fill) is applied host-side during un-permutation.
"""

import hashlib
import sys

import numpy as np

sys.path.insert(0, "/opt/trn_rl_repo")

import concourse.bass as bass  # noqa: E402
import concourse.bacc as bacc  # noqa: E402
import concourse.mybir as mybir  # noqa: E402
import concourse.tile as tile  # noqa: E402

F32 = mybir.dt.float32
F32R = mybir.dt.float32r

# Problem shapes (hardcoded per contest rules)
B, L, NAA, NAT = 32, 4096, 22, 27
N_CORES = 8
BPC = B // N_CORES          # batch rows per core
T = BPC * L                 # tokens per core

# ptab column map
PT_RT = 0                   # 7*16 RT consts: PT_RT + kk*16 + m*4 + j
PT_A1 = 112                 # rot-axis1 ct-coeff 3x3 (i*3+j)
PT_B1 = 121                 # rot-axis1 st-coeff
PT_C1 = 130                 # rot-axis1 const
PT_A2 = 139
PT_B2 = 148
PT_C2 = 157
PT_N = 166


# ----------------------------------------------------------------------------
# host-side tables (mirror reference.py math in float32)
# ----------------------------------------------------------------------------

def make_tables(base_indices, RTs, xyzs_base, mask):
    """Per-type constants. All float32, mirroring reference.py ops."""
    bi = np.asarray(base_indices).astype(np.int64)          # [22,27]
    rt = np.asarray(RTs, np.float32)                        # [22,7,4,4]
    bx = np.asarray(xyzs_base, np.float32)                  # [22,27,4]
    mk = np.asarray(mask).astype(bool)                      # [22,27]

    # ax1/ax2 per type (functions of xyzs_base only) -- float32 math
    NCr = (0.5 * (bx[:, 2, :3] + bx[:, 0, :3])).astype(np.float32)
    CAr = bx[:, 1, :3]
    CBr = bx[:, 4, :3]
    ax1 = np.cross(CBr - CAr, NCr - CAr).astype(np.float32)
    ax1 = ax1 / (np.sqrt((ax1 * ax1).sum(-1, keepdims=True)) + np.float32(1e-8))
    NCp = bx[:, 2, :3] - bx[:, 0, :3]
    NCpp = NCp - ((NCp * NCr).sum(-1, keepdims=True)
                  / (NCr * NCr).sum(-1, keepdims=True)) * NCr
    ax2 = np.cross(CBr - CAr, NCpp).astype(np.float32)
    ax2 = ax2 / (np.sqrt((ax2 * ax2).sum(-1, keepdims=True)) + np.float32(1e-8))

    def axis_coeffs(u):
        # rot_axis(a, u) = ct*A + st*Bm + C  (3x3 blocks)
        uu = np.einsum("si,sj->sij", u, u).astype(np.float32)   # [22,3,3]
        A = np.eye(3, dtype=np.float32)[None] - uu
        C = uu
        Bm = np.zeros((NAA, 3, 3), np.float32)
        Bm[:, 0, 1] = -u[:, 2]; Bm[:, 0, 2] = u[:, 1]
        Bm[:, 1, 0] = u[:, 2];  Bm[:, 1, 2] = -u[:, 0]
        Bm[:, 2, 0] = -u[:, 1]; Bm[:, 2, 1] = u[:, 0]
        return A, Bm, C

    A1, B1, C1 = axis_coeffs(ax1.astype(np.float32))
    A2, B2, C2 = axis_coeffs(ax2.astype(np.float32))

    # einsum matrices: LT[s] maps 108 frame feats -> 81 coords
    # feat r = k*12 + i*4 + j (rows i<3 of each frame k, in the transposed
    # rhs built from rtfs feats [[16,9],[1,12]]); out m = a*3 + i
    LT = np.zeros((NAA, 108, 81), np.float32)
    for s in range(NAA):
        for a in range(NAT):
            k = int(bi[s, a])
            for i in range(3):
                for j in range(4):
                    LT[s, k * 12 + i * 4 + j, a * 3 + i] = bx[s, a, j]

    return dict(rt=rt, A1=A1, B1=B1, C1=C1, A2=A2, B2=B2, C2=C2,
                LT=LT, mask=mk, bi=bi)


# ----------------------------------------------------------------------------
# layout planning (shared across cores -> single SPMD program)
# ----------------------------------------------------------------------------

def make_plan(seq):
    """Partition/column layout shared by all 8 cores."""
    seq = np.asarray(seq).astype(np.int64)
    counts = np.zeros((N_CORES, NAA), np.int64)
    for c in range(N_CORES):
        sc = seq[c * BPC:(c + 1) * BPC].reshape(-1)
        counts[c] = np.bincount(sc, minlength=NAA)
    cmax = counts.max(0)                      # worst-case bucket per type

    W = 136
    while True:
        n_s = np.maximum((cmax + W - 1) // W, (cmax > 0).astype(np.int64))
        if n_s.sum() <= 128:
            break
        W += 8
    n_s = n_s.astype(np.int64)
    p_start = np.concatenate([[0], np.cumsum(n_s)])[:NAA]
    P_used = int(n_s.sum())

    # per-partition type id (unused partitions -> type 0)
    ptype = np.zeros(128, np.int64)
    for s in range(NAA):
        ptype[p_start[s]:p_start[s] + n_s[s]] = s

    return dict(W=int(W), n_s=n_s, p_start=p_start, P_used=P_used,
                ptype=ptype, counts=counts)


def core_slots(plan, seq_core):
    """tokmap [128, W] (token index per slot) + valid mask."""
    W = plan["W"]
    tokmap = np.zeros((128, W), np.int64)
    valid = np.zeros((128, W), bool)
    order = np.argsort(seq_core, kind="stable")
    cnt = np.bincount(seq_core, minlength=NAA)
    off = np.concatenate([[0], np.cumsum(cnt)])
    for s in range(NAA):
        toks = order[off[s]:off[s + 1]]
        ns = int(plan["n_s"][s])
        p0 = int(plan["p_start"][s])
        nslot = ns * W
        if len(toks) == 0:
            continue
        pad = np.full(nslot, toks[0], np.int64)
        pad[:len(toks)] = toks
        tokmap[p0:p0 + ns] = pad.reshape(ns, W)
        v = np.zeros(nslot, bool); v[:len(toks)] = True
        valid[p0:p0 + ns] = v.reshape(ns, W)
    return tokmap, valid


def build_ptab(plan, tables):
    W = plan["W"]; del W
    pt = np.zeros((128, PT_N), np.float32)
    tp = plan["ptype"]
    rt = tables["rt"]
    pt[:, PT_RT:PT_RT + 112] = rt[tp].reshape(128, 112)
    for base, key in ((PT_A1, "A1"), (PT_B1, "B1"), (PT_C1, "C1"),
                      (PT_A2, "A2"), (PT_B2, "B2"), (PT_C2, "C2")):
        pt[:, base:base + 9] = tables[key][tp].reshape(128, 9)
    return pt


def pack_inputs(plan, tokmap, xyz_core, alphas_core):
    """xa [128, 29*W] feature-major: feats 0..8 = N,Ca,C xyz; 9..18 = alpha
    cos comps (pairs 0..9); 19..28 = alpha sin comps."""
    W = plan["W"]
    x9 = xyz_core.reshape(T, 9)
    al = alphas_core.reshape(T, 10, 2)
    xa = np.zeros((128, 29, W), np.float32)
    tm = tokmap
    xa[:, 0:9, :] = x9[tm].transpose(0, 2, 1)
    xa[:, 9:19, :] = al[tm][:, :, :, 0].transpose(0, 2, 1)
    xa[:, 19:29, :] = al[tm][:, :, :, 1].transpose(0, 2, 1)
    return xa.reshape(128, 29 * W)


# ----------------------------------------------------------------------------
# numpy mirror of the device computation (layout-level validation)
# ----------------------------------------------------------------------------

def numpy_mirror(plan, tables, xa):
    """Compute rtfs [128, W, 9, 16] and xyz3s [81, 128*W] like the device."""
    W = plan["W"]
    f = np.float32
    xa = xa.reshape(128, 29, W).astype(f)
    tp = plan["ptype"]
    rt = tables["rt"][tp]          # [128,7,4,4]
    N_, Ca, C_ = xa[:, 0:3], xa[:, 3:6], xa[:, 6:9]
    cq = xa[:, 9:19]; sq = xa[:, 19:29]

    # alpha normalization
    n = np.sqrt(cq * cq + sq * sq).astype(f) + f(1e-6)
    r = (f(1.0) / n).astype(f)
    ct, st = (cq * r).astype(f), (sq * r).astype(f)

    # frame 0
    v1 = C_ - Ca; v2 = N_ - Ca
    r1 = f(1.0) / (np.sqrt((v1 * v1).sum(1)).astype(f) + f(1e-8))
    e1 = v1 * r1[:, None]
    d2 = (e1 * v2).sum(1)
    u2 = v2 - e1 * d2[:, None]
    r2 = f(1.0) / (np.sqrt((u2 * u2).sum(1)).astype(f) + f(1e-8))
    e2 = u2 * r2[:, None]
    e3 = np.cross(e1, e2, axis=1).astype(f)

    RTF = np.zeros((128, W, 9, 4, 4), f)
    R0 = np.stack([e1, e2, e3], axis=2)          # [128,3,3,W] cols e1,e2,e3
    RTF[:, :, 0, :3, :3] = R0.transpose(0, 3, 1, 2)
    RTF[:, :, 0, :3, 3] = Ca.transpose(0, 2, 1)
    RTF[:, :, 0, 3, 3] = 1.0

    def rotX(q):
        m = np.zeros((128, W, 4, 4), f)
        m[..., 0, 0] = 1.0; m[..., 3, 3] = 1.0
        m[..., 1, 1] = ct[:, q]; m[..., 1, 2] = -st[:, q]
        m[..., 2, 1] = st[:, q]; m[..., 2, 2] = ct[:, q]
        return m

    def rotZ(q):
        m = np.zeros((128, W, 4, 4), f)
        m[..., 2, 2] = 1.0; m[..., 3, 3] = 1.0
        m[..., 0, 0] = ct[:, q]; m[..., 0, 1] = -st[:, q]
        m[..., 1, 0] = st[:, q]; m[..., 1, 1] = ct[:, q]
        return m

    def rax(q, An, Bn, Cn):
        A = tables[An][tp]; Bm = tables[Bn][tp]; Cc = tables[Cn][tp]
        m = np.zeros((128, W, 4, 4), f)
        m[..., :3, :3] = (ct[:, q, :, None, None] * A[:, None]
                          + st[:, q, :, None, None] * Bm[:, None]
                          + Cc[:, None]).astype(f)
        m[..., 3, 3] = 1.0
        return m

    mm = lambda a, b: np.matmul(a, b).astype(f)
    rtb = np.broadcast_to(rt[:, None], (128, W, 7, 4, 4))
    RTF[:, :, 1] = mm(mm(RTF[:, :, 0], rtb[:, :, 0]), rotX(0))
    RTF[:, :, 2] = mm(mm(RTF[:, :, 0], rtb[:, :, 1]), rotX(1))
    RTF[:, :, 3] = mm(mm(RTF[:, :, 0], rtb[:, :, 2]), rotX(2))
    RTF[:, :, 8] = mm(mm(RTF[:, :, 0], rax(7, "A1", "B1", "C1")),
                      rax(8, "A2", "B2", "C2"))
    RTF[:, :, 4] = mm(mm(mm(RTF[:, :, 8], rtb[:, :, 3]), rotX(3)), rotZ(9))
    RTF[:, :, 5] = mm(mm(RTF[:, :, 4], rtb[:, :, 4]), rotX(4))
    RTF[:, :, 6] = mm(mm(RTF[:, :, 5], rtb[:, :, 5]), rotX(5))
    RTF[:, :, 7] = mm(mm(RTF[:, :, 6], rtb[:, :, 6]), rotX(6))

    # einsum via per-type LT
    feats = RTF[:, :, :, :3, :].reshape(128, W, 108)     # k*12 + i*4 + j
    LTp = tables["LT"][tp]                               # [128,108,81]
    out = np.matmul(feats, LTp).transpose(2, 0, 1).astype(f)  # [81,128,W]
    return RTF, out.reshape(81, 128 * W)


WC = 48   # einsum column chunk (shared by build_program and unscrambler)


def packed_slot_index(plan):
    """Map packed xyz3s columns back to global slots p*W + w."""
    W = plan["W"]
    idx = []
    col = 0
    while col < W:
        wch = min(WC, W - col)
        for s in range(NAA):
            ns, p0 = int(plan["n_s"][s]), int(plan["p_start"][s])
            if ns == 0:
                continue
            for p in range(p0, p0 + ns):
                base = p * W + col
                idx.extend(range(base, base + wch))
        col += wch
    return np.asarray(idx, np.int64)


def assemble_outputs(plan, tables, seq, rtf_cores, xyz3s_cores,
                     tokmaps, valids):
    """rtf_cores[c]: [P, 144*W]; xyz3s_cores[c]: [81, P*W] packed."""
    W = plan["W"]; P = plan["P_used"]
    pidx = packed_slot_index(plan)
    RTframes = np.zeros((B, L, 9, 4, 4), np.float32)
    xyz3 = np.zeros((B, L, NAT, 3), np.float32)
    for c in range(N_CORES):
        vd = valids[c][:P].reshape(-1)
        toks = tokmaps[c][:P].reshape(-1)[vd]
        arr = rtf_cores[c].reshape(P * W, 144)[vd]
        dst = RTframes[c * BPC:(c + 1) * BPC].reshape(T, 9, 4, 4)
        dst[toks, :, :3, :] = arr[:, :108].reshape(-1, 9, 3, 4)
        dst[toks, :, 3, :] = arr[:, 108:].reshape(-1, 9, 4)
        xs = np.zeros((81, 128 * W), np.float32)
        xs[:, pidx] = xyz3s_cores[c]
        x = xs.reshape(81, 128, W)[:, :P].reshape(81, P * W)
        x = x.T[vd].reshape(-1, NAT, 3)
        xyz3[c * BPC:(c + 1) * BPC].reshape(T, NAT, 3)[toks] = x
    present = tables["mask"][np.asarray(seq).astype(np.int64)]   # [B,L,27]
    xyz3 = np.where(present[..., None], xyz3, np.float32(np.nan))
    return RTframes, xyz3


# ----------------------------------------------------------------------------
# bass program
# ----------------------------------------------------------------------------

def fap(base_ap, off, dims, parts=None):
    """AP with the tile's partition dim + custom free dims ([step,count])."""
    p = list(base_ap.ap[0])
    if parts is not None:
        p = [p[0], parts]
    return bass.AP(base_ap.tensor, base_ap.offset + off,
                   [p] + [list(d) for d in dims])


class Ledger:
    """Greedy engine load balancer for elementwise ops."""

    def __init__(self, nc):
        self.nc = nc
        self.load = {"v": 0.0, "a": 0.0, "g": 0.0}
        self.rate = {"v": 0.95, "a": 0.8, "g": 3.2}
        self.fixed = {"v": 180.0, "a": 300.0, "g": 300.0}

    def pick(self, allowed, fd, g_rate=None):
        best, bc = None, None
        for e in allowed:
            r = self.rate[e] if (e != "g" or g_rate is None) else g_rate
            c = self.load[e] + self.fixed[e] + r * fd
            if bc is None or c < bc:
                best, bc = e, c
        r = self.rate[best] if (best != "g" or g_rate is None) else g_rate
        self.load[best] += self.fixed[best] + r * fd
        return best

    def eng(self, e):
        return {"v": self.nc.vector, "a": self.nc.scalar, "g": self.nc.gpsimd}[e]


def build_program(plan):
    W = plan["W"]
    P = plan["P_used"]
    n_s = plan["n_s"]; p_start = plan["p_start"]
    Wc = WC
    AL = mybir.AluOpType

    nc = bacc.Bacc("TRN2", target_bir_lowering=False, debug=False,
                   num_devices=N_CORES)

    d_xa = nc.dram_tensor("xa", [128, 29 * W], F32, kind="ExternalInput").ap()
    d_pt = nc.dram_tensor("ptab", [128, PT_N], F32, kind="ExternalInput").ap()
    d_lt = nc.dram_tensor("lt", [108, NAA * 81], F32, kind="ExternalInput").ap()
    d_id = nc.dram_tensor("ident", [128, 128], F32, kind="ExternalInput").ap()
    d_rtf = nc.dram_tensor("rtf", [P, 144 * W], F32, kind="ExternalOutput").ap()
    d_x3 = nc.dram_tensor("xyz3s", [81, P * W], F32, kind="ExternalOutput").ap()

    with tile.TileContext(nc) as tc:
        import contextlib
        ctx = contextlib.ExitStack()
        with ctx:
            pool = ctx.enter_context(tc.tile_pool(name="main", bufs=1))
            ps_tr = ctx.enter_context(
                tc.tile_pool(name="ps_tr", bufs=2, space="PSUM"))
            ps_mm = ctx.enter_context(
                tc.tile_pool(name="ps_mm", bufs=2, space="PSUM"))
            xo_pool = ctx.enter_context(tc.tile_pool(name="xo", bufs=3))

            t_xa = pool.tile([128, 29 * W], F32, tag="xa")
            t_rtfs = pool.tile([128, 144 * W], F32, tag="rtfs")
            t_ta = pool.tile([128, 18 * W], F32, tag="ta")
            t_tb = pool.tile([128, 55 * W], F32, tag="tb")
            t_pt = pool.tile([128, PT_N], F32, tag="pt")
            t_lt = pool.tile([128, NAA * 81], F32, tag="lt")
            t_id = pool.tile([128, 128], F32, tag="ident")

            xa = t_xa[:]; rtfs = t_rtfs[:]; ta = t_ta[:]; tb = t_tb[:]
            pt = t_pt[:]; lt = t_lt[:]; idn = t_id[:]
            TR0 = 12 * W                      # TR region inside tb

            led = Ledger(nc)

            # ---------------- input DMAs ----------------
            nc.sync.dma_start(out=xa, in_=d_xa)
            nc.sync.dma_start(out=pt, in_=d_pt)
            nc.sync.dma_start(out=fap(lt, 0, [[1, NAA * 81]], parts=108),
                              in_=d_lt)
            nc.sync.dma_start(out=idn, in_=d_id)

            # AP helpers -----------------------------------------------------
            def xaf(f0, nf=1, bcast=None):
                if bcast is not None:
                    return fap(xa, f0 * W, [[0, bcast], [1, W]])
                return fap(xa, f0 * W, [[W, nf], [1, W]])

            def tbf(f0, nf=1, step=1):
                return fap(tb, f0 * W, [[W * step, nf], [1, W]])

            def tbb(f0, nf):
                return fap(tb, f0 * W, [[0, nf], [1, W]])

            def taf(f0, nf=1):
                return fap(ta, f0 * W, [[W, nf], [1, W]])

            def rf(k, i, j, ni=1):
                """rows0-2 feats: k*12 + i*4 + j, ni rows merged (step 4)."""
                return fap(rtfs, k * 12 + i * 4 + j, [[4, ni], [144, W]])

            def r3f(k, j, nj=1):
                """row3 feats: 108 + k*4 + j."""
                return fap(rtfs, 108 + k * 4 + j, [[1, nj], [144, W]])

            def pts(c):
                return fap(pt, c, [[1, 1]])

            def ptb(c, nf):
                return fap(pt, c, [[1, nf], [0, W]])

            def ew(e_allowed, emit, fd, g_rate=None):
                e = led.pick(e_allowed, fd, g_rate)
                emit(led.eng(e))

            def ts_mul(out, in0, scalar_ap, fd):
                e = led.pick(("v", "a"), fd)
                E = led.eng(e)
                if e == "a":
                    E.mul(out=out, in_=in0, mul=scalar_ap)
                else:
                    E.tensor_scalar(out=out, in0=in0, scalar1=scalar_ap,
                                    scalar2=None, op0=AL.mult)

            def const_write(out, c, fd):
                """out[:] = ptab const broadcast (out = 0*x + c)."""
                e = led.pick(("v", "a"), fd)
                E = led.eng(e)
                if e == "a":
                    E.activation(out=out, in_=xaf(0, 1),
                                 func=mybir.ActivationFunctionType.Identity,
                                 bias=pts(c), scale=0.0)
                else:
                    E.tensor_scalar(out=out, in0=xaf(0, 1), scalar1=0.0,
                                    scalar2=pts(c), op0=AL.mult, op1=AL.add)

            def stt(out, in0, c, fd, op1=AL.add):
                ew(("v",), lambda E: E.scalar_tensor_tensor(
                    out=out, in0=in0, scalar=pts(c), in1=out,
                    op0=AL.mult, op1=op1), fd)

            def tt(out, in0, in1, op, fd):
                ew(VG, lambda E: E.tensor_tensor(
                    out=out, in0=in0, in1=in1, op=op), fd)

            VG = ("v", "g")
            VAG = ("v", "a", "g")
            cA, sA = 9, 19

            def ctf(q, bcast=None):
                return xaf(cA + q, 1) if bcast is None else xaf(cA + q, bcast=bcast)

            def stf(q, bcast=None):
                return xaf(sA + q, 1) if bcast is None else xaf(sA + q, bcast=bcast)

            # ---------------- stage A: alpha normalization ------------------
            tt(tbf(0, 10), xaf(cA, 10), xaf(cA, 10), AL.mult, 10 * W)
            tt(tbf(10, 10), xaf(sA, 10), xaf(sA, 10), AL.mult, 10 * W)
            tt(tbf(0, 10), tbf(0, 10), tbf(10, 10), AL.add, 10 * W)
            nc.scalar.sqrt(out=tbf(10, 10), in_=tbf(0, 10))
            ew(VG, lambda E: E.tensor_scalar(
                out=tbf(10, 10), in0=tbf(10, 10), scalar1=1e-6, scalar2=None,
                op0=AL.add), 10 * W)
            nc.vector.reciprocal(out=tbf(0, 10), in_=tbf(10, 10))
            tt(xaf(cA, 10), xaf(cA, 10), tbf(0, 10), AL.mult, 10 * W)
            tt(xaf(sA, 10), xaf(sA, 10), tbf(0, 10), AL.mult, 10 * W)

            # ---------------- stage B: frame 0 ------------------------------
            tt(tbf(0, 3), xaf(6, 3), xaf(3, 3), AL.subtract, 3 * W)
            tt(tbf(3, 3), xaf(0, 3), xaf(3, 3), AL.subtract, 3 * W)
            tt(tbf(6, 3), tbf(0, 3), tbf(0, 3), AL.mult, 3 * W)
            nc.vector.tensor_reduce(
                out=tbf(9, 1), in_=fap(tb, 6 * W, [[1, W], [W, 3]]),
                axis=mybir.AxisListType.X, op=AL.add)
            nc.scalar.sqrt(out=tbf(10, 1), in_=tbf(9, 1))
            ew(VG, lambda E: E.tensor_scalar(
                out=tbf(10, 1), in0=tbf(10, 1), scalar1=1e-8, scalar2=None,
                op0=AL.add), W)
            nc.vector.reciprocal(out=tbf(9, 1), in_=tbf(10, 1))
            tt(rf(0, 0, 0, ni=3), tbf(0, 3), tbb(9, 3), AL.mult, 3 * W)
            tt(tbf(6, 3), rf(0, 0, 0, ni=3), tbf(3, 3), AL.mult, 3 * W)
            nc.vector.tensor_reduce(
                out=tbf(9, 1), in_=fap(tb, 6 * W, [[1, W], [W, 3]]),
                axis=mybir.AxisListType.X, op=AL.add)
            tt(tbf(6, 3), rf(0, 0, 0, ni=3), tbb(9, 3), AL.mult, 3 * W)
            tt(tbf(0, 3), tbf(3, 3), tbf(6, 3), AL.subtract, 3 * W)
            tt(tbf(6, 3), tbf(0, 3), tbf(0, 3), AL.mult, 3 * W)
            nc.vector.tensor_reduce(
                out=tbf(9, 1), in_=fap(tb, 6 * W, [[1, W], [W, 3]]),
                axis=mybir.AxisListType.X, op=AL.add)
            nc.scalar.sqrt(out=tbf(10, 1), in_=tbf(9, 1))
            ew(VG, lambda E: E.tensor_scalar(
                out=tbf(10, 1), in0=tbf(10, 1), scalar1=1e-8, scalar2=None,
                op0=AL.add), W)
            nc.vector.reciprocal(out=tbf(9, 1), in_=tbf(10, 1))
            tt(rf(0, 0, 1, ni=3), tbf(0, 3), tbb(9, 3), AL.mult, 3 * W)
            for cc in range(3):
                i1, i2 = (cc + 1) % 3, (cc + 2) % 3
                tt(tbf(6, 1), rf(0, i1, 0), rf(0, i2, 1), AL.mult, W)
                tt(tbf(7, 1), rf(0, i2, 0), rf(0, i1, 1), AL.mult, W)
                tt(rf(0, cc, 2), tbf(6, 1), tbf(7, 1), AL.subtract, W)
            nc.scalar.copy(out=rf(0, 0, 3, ni=3), in_=xaf(3, 3))
            nc.vector.memset(fap(rtfs, 108, [[1, 3], [144, W]]), 0.0)
            nc.vector.memset(fap(rtfs, 111, [[144, W]]), 1.0)
            # frame8 row3 = [0,0,0,1]
            nc.gpsimd.memset(fap(rtfs, 140, [[1, 3], [144, W]]), 0.0)
            nc.gpsimd.memset(fap(rtfs, 143, [[144, W]]), 1.0)

            # ---------------- stage C: rot-axis matrices --------------------
            for (q, ba, bb, bc, t0) in ((7, PT_A1, PT_B1, PT_C1, 0),
                                        (8, PT_A2, PT_B2, PT_C2, 9)):
                tt(taf(t0, 9), ctf(q, bcast=9), ptb(ba, 9), AL.mult, 9 * W)
                tt(tbf(12, 9), stf(q, bcast=9), ptb(bb, 9), AL.mult, 9 * W)
                tt(taf(t0, 9), taf(t0, 9), tbf(12, 9), AL.add, 9 * W)
                tt(taf(t0, 9), taf(t0, 9), ptb(bc, 9), AL.add, 9 * W)

            # ---------------- stage D: frame 8 rows0-2 ----------------------
            def p1f(j, ni=3):
                return fap(tb, (21 + j) * W, [[3 * W, ni], [1, W]])

            for (src_t0, dst) in ((0, "p1"), (9, "rtfs8")):
                for j in range(3):
                    for k in range(3):
                        if dst == "p1":
                            in0 = rf(0, 0, k, ni=3)
                            outp = p1f(j)
                        else:
                            in0 = p1f(k)
                            outp = rf(8, 0, j, ni=3)
                        a1 = fap(ta, (src_t0 + k * 3 + j) * W, [[0, 3], [1, W]])
                        if k == 0:
                            tt(outp, in0, a1, AL.mult, 3 * W)
                        else:
                            tt(tbf(30, 3), in0, a1, AL.mult, 3 * W)
                            tt(outp, outp, tbf(30, 3), AL.add, 3 * W)
            nc.scalar.copy(out=rf(8, 0, 3, ni=3), in_=rf(0, 0, 3, ni=3))

            # ------------- chain steps rows0-2 ------------------------------
            def chain_rows(frame, left, kk, q, g0, q0):
                def gcol(j):
                    return fap(tb, (g0 + (j - 1)) * W, [[2 * W, 3], [1, W]])

                for j in range(4):
                    outp = rf(frame, 0, j, ni=3) if j in (0, 3) else gcol(j)
                    for m in range(4):
                        c = PT_RT + kk * 16 + m * 4 + j
                        if m == 0:
                            ts_mul(outp, rf(left, 0, 0, ni=3), pts(c), 3 * W)
                        else:
                            stt(outp, rf(left, 0, m, ni=3), c, 3 * W)
                # rotX cols 1,2
                tt(rf(frame, 0, 1, ni=3), gcol(1), ctf(q, bcast=3), AL.mult, 3 * W)
                tt(tbf(q0, 3), gcol(2), stf(q, bcast=3), AL.mult, 3 * W)
                tt(rf(frame, 0, 1, ni=3), rf(frame, 0, 1, ni=3), tbf(q0, 3),
                   AL.add, 3 * W)
                tt(rf(frame, 0, 2, ni=3), gcol(2), ctf(q, bcast=3), AL.mult, 3 * W)
                tt(tbf(q0, 3), gcol(1), stf(q, bcast=3), AL.mult, 3 * W)
                tt(rf(frame, 0, 2, ni=3), rf(frame, 0, 2, ni=3), tbf(q0, 3),
                   AL.subtract, 3 * W)

            def chain_row3_const(frame, kk, q, tmp):
                """left row3 == [0,0,0,1]: G row3 = RT[kk] row3 (consts)."""
                c3 = lambda j: PT_RT + kk * 16 + 12 + j
                const_write(r3f(frame, 0), c3(0), W)
                const_write(r3f(frame, 3), c3(3), W)
                ts_mul(r3f(frame, 1), ctf(q), pts(c3(1)), W)
                stt(r3f(frame, 1), stf(q), c3(2), W)
                ts_mul(r3f(frame, 2), ctf(q), pts(c3(2)), W)
                ts_mul(tbf(tmp, 1), stf(q), pts(c3(1)), W)
                tt(r3f(frame, 2), r3f(frame, 2), tbf(tmp, 1), AL.subtract, W)

            def chain_row3_full(frame, left, kk, q, g0, tmp):
                """left row3 per-token: full G + rotX on row3."""
                c = lambda m, j: PT_RT + kk * 16 + m * 4 + j
                stage = {1: tbf(g0, 1), 2: tbf(g0 + 1, 1)}
                for j in range(4):
                    outp = r3f(frame, j) if j in (0, 3) else stage[j]
                    ts_mul(outp, r3f(left, 0), pts(c(0, j)), W)
                    for m in range(1, 4):
                        stt(outp, r3f(left, m), c(m, j), W)
                tt(r3f(frame, 1), stage[1], ctf(q), AL.mult, W)
                tt(tbf(tmp, 1), stage[2], stf(q), AL.mult, W)
                tt(r3f(frame, 1), r3f(frame, 1), tbf(tmp, 1), AL.add, W)
                tt(r3f(frame, 2), stage[2], ctf(q), AL.mult, W)
                tt(tbf(tmp, 1), stage[1], stf(q), AL.mult, W)
                tt(r3f(frame, 2), r3f(frame, 2), tbf(tmp, 1), AL.subtract, W)

            chain_rows(1, 0, 0, 0, g0=33, q0=39)
            chain_row3_const(1, 0, 0, tmp=42)
            chain_rows(2, 0, 1, 1, g0=43, q0=49)
            chain_row3_const(2, 1, 1, tmp=52)
            chain_rows(3, 0, 2, 2, g0=0, q0=6)
            chain_row3_const(3, 2, 2, tmp=9)

            # ---------------- frame 4 rows0-2 -------------------------------
            def g3col(j):
                return fap(tb, (21 + j) * W, [[3 * W, 3], [1, W]])

            for j in range(4):
                outp = rf(4, 0, j, ni=3) if j == 3 else g3col(j)
                for m in range(4):
                    c = PT_RT + 3 * 16 + m * 4 + j
                    if m == 0:
                        ts_mul(outp, rf(8, 0, 0, ni=3), pts(c), 3 * W)
                    else:
                        stt(outp, rf(8, 0, m, ni=3), c, 3 * W)
            # H1 = g1*ct3 + g2*st3 (tb 30-33); col2 = g2*ct3 - g1*st3
            tt(tbf(30, 3), g3col(1), ctf(3, bcast=3), AL.mult, 3 * W)
            tt(tbf(12, 3), g3col(2), stf(3, bcast=3), AL.mult, 3 * W)
            tt(tbf(30, 3), tbf(30, 3), tbf(12, 3), AL.add, 3 * W)
            tt(rf(4, 0, 2, ni=3), g3col(2), ctf(3, bcast=3), AL.mult, 3 * W)
            tt(tbf(12, 3), g3col(1), stf(3, bcast=3), AL.mult, 3 * W)
            tt(rf(4, 0, 2, ni=3), rf(4, 0, 2, ni=3), tbf(12, 3), AL.subtract,
               3 * W)
            # rotZ9: col0 = g0*c9 + H1*s9 ; col1 = H1*c9 - g0*s9
            tt(rf(4, 0, 0, ni=3), g3col(0), ctf(9, bcast=3), AL.mult, 3 * W)
            tt(tbf(12, 3), tbf(30, 3), stf(9, bcast=3), AL.mult, 3 * W)
            tt(rf(4, 0, 0, ni=3), rf(4, 0, 0, ni=3), tbf(12, 3), AL.add, 3 * W)
            tt(rf(4, 0, 1, ni=3), tbf(30, 3), ctf(9, bcast=3), AL.mult, 3 * W)
            tt(tbf(12, 3), g3col(0), stf(9, bcast=3), AL.mult, 3 * W)
            tt(rf(4, 0, 1, ni=3), rf(4, 0, 1, ni=3), tbf(12, 3), AL.subtract,
               3 * W)
            # frame 4 row3: rt3row3 @ rotX3 @ rotZ9 (rt3row3 const)
            c3 = lambda j: PT_RT + 3 * 16 + 12 + j
            ts_mul(tbf(15, 1), ctf(3), pts(c3(1)), W)       # a1
            stt(tbf(15, 1), stf(3), c3(2), W)
            ts_mul(r3f(4, 2), ctf(3), pts(c3(2)), W)        # a2 direct
            ts_mul(tbf(16, 1), stf(3), pts(c3(1)), W)
            tt(r3f(4, 2), r3f(4, 2), tbf(16, 1), AL.subtract, W)
            const_write(r3f(4, 3), c3(3), W)
            ts_mul(r3f(4, 0), ctf(9), pts(c3(0)), W)        # a0*c9
            tt(tbf(16, 1), tbf(15, 1), stf(9), AL.mult, W)  # a1*s9
            tt(r3f(4, 0), r3f(4, 0), tbf(16, 1), AL.add, W)
            tt(r3f(4, 1), tbf(15, 1), ctf(9), AL.mult, W)   # a1*c9
            ts_mul(tbf(16, 1), stf(9), pts(c3(0)), W)       # a0*s9
            tt(r3f(4, 1), r3f(4, 1), tbf(16, 1), AL.subtract, W)

            for (fr_i, left, kk, q) in ((5, 4, 4, 4), (6, 5, 5, 5), (7, 6, 6, 6)):
                chain_rows(fr_i, left, kk, q, g0=21, q0=27)
                chain_row3_full(fr_i, left, kk, q, g0=30, tmp=32)

            # ---------------- frame output DMA (contiguous) -----------------
            nc.sync.dma_start(
                out=d_rtf, in_=fap(rtfs, 0, [[1, 144 * W]], parts=P))

            # ---------------- einsum ----------------------------------------
            packed_off = 0
            col = 0
            while col < W:
                wch = min(Wc, W - col)
                ngrp = (wch + 3) // 4
                for g in range(ngrp):
                    pst = ps_tr.tile([128, 512], F32, tag="pstr")
                    nw = min(4, wch - g * 4)
                    for r in range(nw):
                        w = col + g * 4 + r
                        nc.tensor.matmul(
                            out=fap(pst[:], r * 128, [[1, 128]], parts=108),
                            lhsT=fap(rtfs, w * 144, [[1, 108]]),
                            rhs=idn, is_transpose=True,
                            start=True, stop=True)
                    # scatter-copy psum -> TR (tokens contiguous per type)
                    e = led.pick(("v", "a"), nw * 128)
                    cp_out = fap(tb, TR0 + g * 4, [[1, nw], [wch, 128]],
                                 parts=108)
                    cp_in = fap(pst[:], 0, [[128, nw], [1, 128]], parts=108)
                    if e == "a":
                        led.eng(e).copy(out=cp_out, in_=cp_in)
                    else:
                        led.eng(e).tensor_copy(out=cp_out, in_=cp_in)
                for s in range(NAA):
                    ns = int(n_s[s]); p0 = int(p_start[s])
                    if ns == 0:
                        continue
                    N = ns * wch
                    psm = ps_mm.tile([128, 512], F32, tag="psmm")
                    nc.tensor.matmul(
                        out=fap(psm[:], 0, [[1, N]], parts=81),
                        lhsT=fap(lt, s * 81, [[1, 81]], parts=108),
                        rhs=fap(tb, TR0 + p0 * wch, [[1, N]], parts=108),
                        start=True, stop=True)
                    xo = xo_pool.tile([128, 512], F32, tag="xo")
                    e = led.pick(("v", "a"), N)
                    xo_out = fap(xo[:], 0, [[1, N]], parts=81)
                    xo_in = fap(psm[:], 0, [[1, N]], parts=81)
                    if e == "a":
                        led.eng(e).copy(out=xo_out, in_=xo_in)
                    else:
                        led.eng(e).tensor_copy(out=xo_out, in_=xo_in)
                    nc.sync.dma_start(
                        out=bass.AP(d_x3.tensor, packed_off,
                                    [[P * W, 81], [1, N]]),
                        in_=fap(xo[:], 0, [[1, N]], parts=81))
                    packed_off += N
                col += wch

    nc.compile()
    return nc


# ----------------------------------------------------------------------------
# kernel entry
# ----------------------------------------------------------------------------

_CACHE = {}


def kernel(seq, xyz, alphas, base_indices, RTs_in_base_frame,
           xyzs_in_base_frame, allatom_mask):
    seq = np.asarray(seq).astype(np.int64)
    xyz = np.asarray(xyz, np.float32)
    alphas = np.asarray(alphas, np.float32)

    key = hashlib.sha1(
        seq.tobytes() + np.asarray(base_indices).astype(np.int64).tobytes()
        + np.asarray(RTs_in_base_frame, np.float32).tobytes()
        + np.asarray(xyzs_in_base_frame, np.float32).tobytes()
        + np.asarray(allatom_mask).astype(np.uint8).tobytes()).hexdigest()

    if key in _CACHE:
        plan, tables, nc, tokmaps, valids = _CACHE[key]
    else:
        tables = make_tables(base_indices, RTs_in_base_frame,
                             xyzs_in_base_frame, allatom_mask)
        plan = make_plan(seq)
        tokmaps, valids = [], []
        for c in range(N_CORES):
            tm, vd = core_slots(plan, seq[c * BPC:(c + 1) * BPC].reshape(-1))
            tokmaps.append(tm); valids.append(vd)
        nc = build_program(plan)
        _CACHE[key] = (plan, tables, nc, tokmaps, valids)

    ptab = build_ptab(plan, tables)
    ltp = np.ascontiguousarray(
        tables["LT"].transpose(1, 0, 2).reshape(108, NAA * 81))
    ident = np.eye(128, dtype=np.float32)

    in_maps = []
    for c in range(N_CORES):
        xa = pack_inputs(plan, tokmaps[c],
                         xyz[c * BPC:(c + 1) * BPC],
                         alphas[c * BPC:(c + 1) * BPC])
        in_maps.append({"xa": xa, "ptab": ptab, "lt": ltp, "ident": ident})

    from concourse.bass_utils import run_bass_kernel_spmd
    global _LAST
    _LAST = (nc, in_maps)
    res = run_bass_kernel_spmd(nc, in_maps, core_ids=list(range(N_CORES)))

    rtf_cores = [res.results[c]["rtf"] for c in range(N_CORES)]
    xyz3s_cores = [res.results[c]["xyz3s"] for c in range(N_CORES)]
    return assemble_outputs(plan, tables, seq, rtf_cores, xyz3s_cores,
                            tokmaps, valids)


_LAST = None


def _ensure_ntff_hook():
    """Shim antenv.axon_hooks if the image lacks it (boot degrades silently)."""
    import types
    try:
        from antenv.axon_hooks import get_axon_ntff_profile_hook  # noqa: F401
        return
    except ImportError:
        pass
    import antenv
    from trn_agent_boot.trn_boot import _ntff_profile_via_ctypes
    hook = [_ntff_profile_via_ctypes("/opt/axon/libaxon_pjrt.so")]
    mod = types.ModuleType("antenv.axon_hooks")
    mod.get_axon_ntff_profile_hook = lambda: hook[0]
    mod.set_axon_ntff_profile_hook = lambda h: hook.__setitem__(0, h)
    sys.modules["antenv.axon_hooks"] = mod
    antenv.axon_hooks = mod


def bench_hw(trace=True):
    """Re-run the last-compiled program with NTFF tracing for HW timing."""
    if _LAST is None:
        return None
    if trace:
        _ensure_ntff_hook()
    from concourse.bass_utils import run_bass_kernel_spmd
    nc, in_maps = _LAST
    return run_bass_kernel_spmd(nc, in_maps, core_ids=list(range(N_CORES)),
                                trace=trace)


# revision 25
# speedup vs baseline: 1.9886x; 1.1240x over previous
"""Trainium2 Bass kernel for ComputeAllAtomCoords.

Strategy (see problem spec: B=32, L=4096, 22 aa types, 27 atoms, 8 cores):
  - Shard batch across 8 cores (4 batch rows / core, T=16384 tokens each).
  - Host sorts each core's tokens by aa type (seq value). All per-type
    tables (RT matrices, rot-axis consts, base coords, frame-selection
    indices, atom masks) become piecewise-constant along the sorted
    stream. Tokens are laid out [128 partitions x W columns] with every
    partition holding tokens of a single type, so per-type constants are
    [P,1] per-partition scalars -> fused 1-op MACs (scalar_tensor_tensor)
    on the Vector/GpSimd engines and scale/bias operands on Scalar engine.
  - The full 4x4-matrix frame chain (RTF0..RTF8) is computed SoA style
    (tokens across partitions+free dim) with merged 4-row strided APs.
  - The final "gather frame by base_indices and apply to base coords"
    einsum collapses, per type, into a fixed [108 -> 81] linear map of the
    token's flattened frames. Tokens of one type occupy a contiguous
    partition range, so after a PE transpose (token-major -> feature-major)
    it becomes a plain matmul per type with the per-type matrix baked on
    the host. fp32r at N>=256 runs at full PE rate.
  - Atom masking (NaN fill) is applied host-side during un-permutation.
"""

import hashlib
import sys

import numpy as np

sys.path.insert(0, "/opt/trn_rl_repo")

import concourse.bass as bass  # noqa: E402
import concourse.bacc as bacc  # noqa: E402
import concourse.mybir as mybir  # noqa: E402
import concourse.tile as tile  # noqa: E402

F32 = mybir.dt.float32
F32R = mybir.dt.float32r

# Problem shapes (hardcoded per contest rules)
B, L, NAA, NAT = 32, 4096, 22, 27
N_CORES = 8
BPC = B // N_CORES          # batch rows per core
T = BPC * L                 # tokens per core

# ptab column map
PT_RT = 0                   # 7*16 RT consts: PT_RT + kk*16 + m*4 + j
PT_A1 = 112                 # rot-axis1 ct-coeff 3x3 (i*3+j)
PT_B1 = 121                 # rot-axis1 st-coeff
PT_C1 = 130                 # rot-axis1 const
PT_A2 = 139
PT_B2 = 148
PT_C2 = 157
PT_N = 166


# ----------------------------------------------------------------------------
# host-side tables (mirror reference.py math in float32)
# ----------------------------------------------------------------------------

def make_tables(base_indices, RTs, xyzs_base, mask):
    """Per-type constants. All float32, mirroring reference.py ops."""
    bi = np.asarray(base_indices).astype(np.int64)          # [22,27]
    rt = np.asarray(RTs, np.float32)                        # [22,7,4,4]
    bx = np.asarray(xyzs_base, np.float32)                  # [22,27,4]
    mk = np.asarray(mask).astype(bool)                      # [22,27]

    # ax1/ax2 per type (functions of xyzs_base only) -- float32 math
    NCr = (0.5 * (bx[:, 2, :3] + bx[:, 0, :3])).astype(np.float32)
    CAr = bx[:, 1, :3]
    CBr = bx[:, 4, :3]
    ax1 = np.cross(CBr - CAr, NCr - CAr).astype(np.float32)
    ax1 = ax1 / (np.sqrt((ax1 * ax1).sum(-1, keepdims=True)) + np.float32(1e-8))
    NCp = bx[:, 2, :3] - bx[:, 0, :3]
    NCpp = NCp - ((NCp * NCr).sum(-1, keepdims=True)
                  / (NCr * NCr).sum(-1, keepdims=True)) * NCr
    ax2 = np.cross(CBr - CAr, NCpp).astype(np.float32)
    ax2 = ax2 / (np.sqrt((ax2 * ax2).sum(-1, keepdims=True)) + np.float32(1e-8))

    def axis_coeffs(u):
        # rot_axis(a, u) = ct*A + st*Bm + C  (3x3 blocks)
        uu = np.einsum("si,sj->sij", u, u).astype(np.float32)   # [22,3,3]
        A = np.eye(3, dtype=np.float32)[None] - uu
        C = uu
        Bm = np.zeros((NAA, 3, 3), np.float32)
        Bm[:, 0, 1] = -u[:, 2]; Bm[:, 0, 2] = u[:, 1]
        Bm[:, 1, 0] = u[:, 2];  Bm[:, 1, 2] = -u[:, 0]
        Bm[:, 2, 0] = -u[:, 1]; Bm[:, 2, 1] = u[:, 0]
        return A, Bm, C

    A1, B1, C1 = axis_coeffs(ax1.astype(np.float32))
    A2, B2, C2 = axis_coeffs(ax2.astype(np.float32))

    # einsum matrices: LT[s] maps 108 frame feats -> 81 coords
    # feat r = k*12 + i*4 + j (rows i<3 of each frame k, in the transposed
    # rhs built from rtfs feats [[16,9],[1,12]]); out m = a*3 + i
    LT = np.zeros((NAA, 108, 81), np.float32)
    for s in range(NAA):
        for a in range(NAT):
            k = int(bi[s, a])
            for i in range(3):
                for j in range(4):
                    LT[s, k * 12 + i * 4 + j, a * 3 + i] = bx[s, a, j]

    return dict(rt=rt, A1=A1, B1=B1, C1=C1, A2=A2, B2=B2, C2=C2,
                LT=LT, mask=mk, bi=bi)


# ----------------------------------------------------------------------------
# layout planning (shared across cores -> single SPMD program)
# ----------------------------------------------------------------------------

def make_plan(seq):
    """Partition/column layout shared by all 8 cores."""
    seq = np.asarray(seq).astype(np.int64)
    counts = np.zeros((N_CORES, NAA), np.int64)
    for c in range(N_CORES):
        sc = seq[c * BPC:(c + 1) * BPC].reshape(-1)
        counts[c] = np.bincount(sc, minlength=NAA)
    cmax = counts.max(0)                      # worst-case bucket per type

    W = 136
    while True:
        n_s = np.maximum((cmax + W - 1) // W, (cmax > 0).astype(np.int64))
        if n_s.sum() <= 128:
            break
        W += 8
    n_s = n_s.astype(np.int64)
    p_start = np.concatenate([[0], np.cumsum(n_s)])[:NAA]
    P_used = int(n_s.sum())

    # per-partition type id (unused partitions -> type 0)
    ptype = np.zeros(128, np.int64)
    for s in range(NAA):
        ptype[p_start[s]:p_start[s] + n_s[s]] = s

    return dict(W=int(W), n_s=n_s, p_start=p_start, P_used=P_used,
                ptype=ptype, counts=counts)


def core_slots(plan, seq_core):
    """tokmap [128, W] (token index per slot) + valid mask."""
    W = plan["W"]
    tokmap = np.zeros((128, W), np.int64)
    valid = np.zeros((128, W), bool)
    order = np.argsort(seq_core, kind="stable")
    cnt = np.bincount(seq_core, minlength=NAA)
    off = np.concatenate([[0], np.cumsum(cnt)])
    for s in range(NAA):
        toks = order[off[s]:off[s + 1]]
        ns = int(plan["n_s"][s])
        p0 = int(plan["p_start"][s])
        nslot = ns * W
        if len(toks) == 0:
            continue
        pad = np.full(nslot, toks[0], np.int64)
        pad[:len(toks)] = toks
        tokmap[p0:p0 + ns] = pad.reshape(ns, W)
        v = np.zeros(nslot, bool); v[:len(toks)] = True
        valid[p0:p0 + ns] = v.reshape(ns, W)
    return tokmap, valid


def build_ptab(plan, tables):
    W = plan["W"]; del W
    pt = np.zeros((128, PT_N), np.float32)
    tp = plan["ptype"]
    rt = tables["rt"]
    pt[:, PT_RT:PT_RT + 112] = rt[tp].reshape(128, 112)
    for base, key in ((PT_A1, "A1"), (PT_B1, "B1"), (PT_C1, "C1"),
                      (PT_A2, "A2"), (PT_B2, "B2"), (PT_C2, "C2")):
        pt[:, base:base + 9] = tables[key][tp].reshape(128, 9)
    return pt


def pack_inputs(plan, tokmap, xyz_core, alphas_core):
    """xa [128, 29*W] feature-major: feats 0..8 = N,Ca,C xyz; 9..18 = alpha
    cos comps (pairs 0..9); 19..28 = alpha sin comps."""
    W = plan["W"]
    x9 = xyz_core.reshape(T, 9)
    al = alphas_core.reshape(T, 10, 2)
    xa = np.zeros((128, 29, W), np.float32)
    tm = tokmap
    xa[:, 0:9, :] = x9[tm].transpose(0, 2, 1)
    xa[:, 9:19, :] = al[tm][:, :, :, 0].transpose(0, 2, 1)
    xa[:, 19:29, :] = al[tm][:, :, :, 1].transpose(0, 2, 1)
    return xa.reshape(128, 29 * W)


# ----------------------------------------------------------------------------
# numpy mirror of the device computation (layout-level validation)
# ----------------------------------------------------------------------------

def numpy_mirror(plan, tables, xa):
    """Compute rtfs [128, W, 9, 16] and xyz3s [81, 128*W] like the device."""
    W = plan["W"]
    f = np.float32
    xa = xa.reshape(128, 29, W).astype(f)
    tp = plan["ptype"]
    rt = tables["rt"][tp]          # [128,7,4,4]
    N_, Ca, C_ = xa[:, 0:3], xa[:, 3:6], xa[:, 6:9]
    cq = xa[:, 9:19]; sq = xa[:, 19:29]

    # alpha normalization
    n = np.sqrt(cq * cq + sq * sq).astype(f) + f(1e-6)
    r = (f(1.0) / n).astype(f)
    ct, st = (cq * r).astype(f), (sq * r).astype(f)

    # frame 0
    v1 = C_ - Ca; v2 = N_ - Ca
    r1 = f(1.0) / (np.sqrt((v1 * v1).sum(1)).astype(f) + f(1e-8))
    e1 = v1 * r1[:, None]
    d2 = (e1 * v2).sum(1)
    u2 = v2 - e1 * d2[:, None]
    r2 = f(1.0) / (np.sqrt((u2 * u2).sum(1)).astype(f) + f(1e-8))
    e2 = u2 * r2[:, None]
    e3 = np.cross(e1, e2, axis=1).astype(f)

    RTF = np.zeros((128, W, 9, 4, 4), f)
    R0 = np.stack([e1, e2, e3], axis=2)          # [128,3,3,W] cols e1,e2,e3
    RTF[:, :, 0, :3, :3] = R0.transpose(0, 3, 1, 2)
    RTF[:, :, 0, :3, 3] = Ca.transpose(0, 2, 1)
    RTF[:, :, 0, 3, 3] = 1.0

    def rotX(q):
        m = np.zeros((128, W, 4, 4), f)
        m[..., 0, 0] = 1.0; m[..., 3, 3] = 1.0
        m[..., 1, 1] = ct[:, q]; m[..., 1, 2] = -st[:, q]
        m[..., 2, 1] = st[:, q]; m[..., 2, 2] = ct[:, q]
        return m

    def rotZ(q):
        m = np.zeros((128, W, 4, 4), f)
        m[..., 2, 2] = 1.0; m[..., 3, 3] = 1.0
        m[..., 0, 0] = ct[:, q]; m[..., 0, 1] = -st[:, q]
        m[..., 1, 0] = st[:, q]; m[..., 1, 1] = ct[:, q]
        return m

    def rax(q, An, Bn, Cn):
        A = tables[An][tp]; Bm = tables[Bn][tp]; Cc = tables[Cn][tp]
        m = np.zeros((128, W, 4, 4), f)
        m[..., :3, :3] = (ct[:, q, :, None, None] * A[:, None]
                          + st[:, q, :, None, None] * Bm[:, None]
                          + Cc[:, None]).astype(f)
        m[..., 3, 3] = 1.0
        return m

    mm = lambda a, b: np.matmul(a, b).astype(f)
    rtb = np.broadcast_to(rt[:, None], (128, W, 7, 4, 4))
    RTF[:, :, 1] = mm(mm(RTF[:, :, 0], rtb[:, :, 0]), rotX(0))
    RTF[:, :, 2] = mm(mm(RTF[:, :, 0], rtb[:, :, 1]), rotX(1))
    RTF[:, :, 3] = mm(mm(RTF[:, :, 0], rtb[:, :, 2]), rotX(2))
    RTF[:, :, 8] = mm(mm(RTF[:, :, 0], rax(7, "A1", "B1", "C1")),
                      rax(8, "A2", "B2", "C2"))
    RTF[:, :, 4] = mm(mm(mm(RTF[:, :, 8], rtb[:, :, 3]), rotX(3)), rotZ(9))
    RTF[:, :, 5] = mm(mm(RTF[:, :, 4], rtb[:, :, 4]), rotX(4))
    RTF[:, :, 6] = mm(mm(RTF[:, :, 5], rtb[:, :, 5]), rotX(5))
    RTF[:, :, 7] = mm(mm(RTF[:, :, 6], rtb[:, :, 6]), rotX(6))

    # einsum via per-type LT
    feats = RTF[:, :, :, :3, :].reshape(128, W, 108)     # k*12 + i*4 + j
    LTp = tables["LT"][tp]                               # [128,108,81]
    out = np.matmul(feats, LTp).transpose(2, 0, 1).astype(f)  # [81,128,W]
    return RTF, out.reshape(81, 128 * W)


WC = 48   # einsum column chunk (shared by build_program and unscrambler)


def packed_slot_index(plan):
    """Map packed xyz3s columns back to global slots p*W + w."""
    W = plan["W"]
    idx = []
    col = 0
    while col < W:
        wch = min(WC, W - col)
        for s in range(NAA):
            ns, p0 = int(plan["n_s"][s]), int(plan["p_start"][s])
            if ns == 0:
                continue
            for p in range(p0, p0 + ns):
                base = p * W + col
                idx.extend(range(base, base + wch))
        col += wch
    return np.asarray(idx, np.int64)


def assemble_outputs(plan, tables, seq, rtf_cores, xyz3s_cores,
                     tokmaps, valids):
    """rtf_cores[c]: [P, 144*W]; xyz3s_cores[c]: [81, P*W] packed."""
    W = plan["W"]; P = plan["P_used"]
    pidx = packed_slot_index(plan)
    RTframes = np.zeros((B, L, 9, 4, 4), np.float32)
    xyz3 = np.zeros((B, L, NAT, 3), np.float32)
    for c in range(N_CORES):
        vd = valids[c][:P].reshape(-1)
        toks = tokmaps[c][:P].reshape(-1)[vd]
        arr = rtf_cores[c].reshape(P, 144, W).transpose(0, 2, 1)
        arr = arr.reshape(P * W, 144)[vd]
        dst = RTframes[c * BPC:(c + 1) * BPC].reshape(T, 9, 4, 4)
        dst[toks, :, :3, :] = arr[:, :108].reshape(-1, 9, 3, 4)
        dst[toks, :, 3, :] = arr[:, 108:].reshape(-1, 9, 4)
        xs = np.zeros((81, 128 * W), np.float32)
        xs[:, pidx] = xyz3s_cores[c]
        x = xs.reshape(81, 128, W)[:, :P].reshape(81, P * W)
        x = x.T[vd].reshape(-1, NAT, 3)
        xyz3[c * BPC:(c + 1) * BPC].reshape(T, NAT, 3)[toks] = x
    present = tables["mask"][np.asarray(seq).astype(np.int64)]   # [B,L,27]
    xyz3 = np.where(present[..., None], xyz3, np.float32(np.nan))
    return RTframes, xyz3


# ----------------------------------------------------------------------------
# bass program
# ----------------------------------------------------------------------------

def fap(base_ap, off, dims, parts=None):
    """AP with the tile's partition dim + custom free dims ([step,count])."""
    p = list(base_ap.ap[0])
    if parts is not None:
        p = [p[0], parts]
    return bass.AP(base_ap.tensor, base_ap.offset + off,
                   [p] + [list(d) for d in dims])


class Ledger:
    """Greedy engine load balancer for elementwise ops."""

    def __init__(self, nc):
        self.nc = nc
        self.load = {"v": 0.0, "a": 0.0, "g": 0.0}
        self.rate = {"v": 0.95, "a": 0.8, "g": 3.2}
        self.fixed = {"v": 180.0, "a": 300.0, "g": 300.0}

    def pick(self, allowed, fd, g_rate=None):
        best, bc = None, None
        for e in allowed:
            r = self.rate[e] if (e != "g" or g_rate is None) else g_rate
            c = self.load[e] + self.fixed[e] + r * fd
            if bc is None or c < bc:
                best, bc = e, c
        r = self.rate[best] if (best != "g" or g_rate is None) else g_rate
        self.load[best] += self.fixed[best] + r * fd
        return best

    def eng(self, e):
        return {"v": self.nc.vector, "a": self.nc.scalar, "g": self.nc.gpsimd}[e]


def build_program(plan):
    W = plan["W"]
    P = plan["P_used"]
    n_s = plan["n_s"]; p_start = plan["p_start"]
    Wc = WC
    AL = mybir.AluOpType

    nc = bacc.Bacc("TRN2", target_bir_lowering=False, debug=False,
                   num_devices=N_CORES)

    d_xa = nc.dram_tensor("xa", [128, 29 * W], F32, kind="ExternalInput").ap()
    d_pt = nc.dram_tensor("ptab", [128, PT_N], F32, kind="ExternalInput").ap()
    d_lt = nc.dram_tensor("lt", [108, NAA * 81], F32, kind="ExternalInput").ap()
    d_id = nc.dram_tensor("ident", [128, 128], F32, kind="ExternalInput").ap()
    d_rtf = nc.dram_tensor("rtf", [P, 144 * W], F32, kind="ExternalOutput").ap()
    d_x3 = nc.dram_tensor("xyz3s", [81, P * W], F32, kind="ExternalOutput").ap()

    with tile.TileContext(nc) as tc:
        import contextlib
        ctx = contextlib.ExitStack()
        with ctx:
            pool = ctx.enter_context(tc.tile_pool(name="main", bufs=1))
            ps_tr = ctx.enter_context(
                tc.tile_pool(name="ps_tr", bufs=2, space="PSUM"))
            ps_mm = ctx.enter_context(
                tc.tile_pool(name="ps_mm", bufs=2, space="PSUM"))
            xo_pool = ctx.enter_context(tc.tile_pool(name="xo", bufs=3))

            t_xa = pool.tile([128, 29 * W], F32, tag="xa")
            t_rtfs = pool.tile([128, 144 * W], F32, tag="rtfs")
            t_ta = pool.tile([128, 18 * W], F32, tag="ta")
            t_tb = pool.tile([128, 80 * W], F32, tag="tb")
            t_pt = pool.tile([128, PT_N], F32, tag="pt")
            t_lt = pool.tile([128, NAA * 81], F32, tag="lt")
            t_id = pool.tile([128, 128], F32, tag="ident")

            xa = t_xa[:]; rtfs = t_rtfs[:]; ta = t_ta[:]; tb = t_tb[:]
            pt = t_pt[:]; lt = t_lt[:]; idn = t_id[:]
            TR0 = 20 * W                      # TR region inside tb (elems)

            led = Ledger(nc)

            # ---------------- input DMAs ----------------
            nc.sync.dma_start(out=xa, in_=d_xa)
            nc.sync.dma_start(out=pt, in_=d_pt)
            nc.sync.dma_start(out=fap(lt, 0, [[1, NAA * 81]], parts=108),
                              in_=d_lt)
            nc.sync.dma_start(out=idn, in_=d_id)

            # AP helpers (all feature-major: col = feat*W + w) --------------
            def xaf(f0, nf=1, bcast=None):
                if bcast is not None:
                    return fap(xa, f0 * W, [[0, bcast], [1, W]])
                return fap(xa, f0 * W, [[W, nf], [1, W]])

            def tbf(f0, nf=1):
                return fap(tb, f0 * W, [[W, nf], [1, W]])

            def tb1(f0, nf=1):          # fully-1D contiguous view
                return fap(tb, f0 * W, [[1, nf * W]])

            def tbb(f0, nf):
                return fap(tb, f0 * W, [[0, nf], [1, W]])

            def taf(f0, nf=1):
                return fap(ta, f0 * W, [[W, nf], [1, W]])

            def rff(f0, nf=1, fstep=1):
                return fap(rtfs, f0 * W, [[fstep * W, nf], [1, W]])

            def rf(k, i, j, ni=1):
                """frame rows0-2 feats: k*12 + i*4 + j (row step 4 feats)."""
                return rff(k * 12 + i * 4 + j, ni, fstep=4)

            def r3f(k, j, nj=1):
                return rff(108 + k * 4 + j, nj)

            def pts(c):
                return fap(pt, c, [[1, 1]])

            def ptb(c, nf):
                return fap(pt, c, [[1, nf], [0, W]])

            def ew(e_allowed, emit, fd, g_rate=None):
                e = led.pick(e_allowed, fd, g_rate)
                emit(led.eng(e))

            VG = ("v", "g")
            V = ("v",)
            VAG = ("v", "a", "g")

            def ts_mul(out, in0, scalar_ap, fd):
                e = led.pick(("v", "a"), fd)
                E = led.eng(e)
                if e == "a":
                    E.mul(out=out, in_=in0, mul=scalar_ap)
                else:
                    E.tensor_scalar(out=out, in0=in0, scalar1=scalar_ap,
                                    scalar2=None, op0=AL.mult)

            def const_write(out, c, fd):
                e = led.pick(("v", "a"), fd)
                E = led.eng(e)
                if e == "a":
                    E.activation(out=out, in_=xaf(0, 1),
                                 func=mybir.ActivationFunctionType.Identity,
                                 bias=pts(c), scale=0.0)
                else:
                    E.tensor_scalar(out=out, in0=xaf(0, 1), scalar1=0.0,
                                    scalar2=pts(c), op0=AL.mult, op1=AL.add)

            def stt(out, in0, c, fd, op1=AL.add):
                ew(V, lambda E: E.scalar_tensor_tensor(
                    out=out, in0=in0, scalar=pts(c), in1=out,
                    op0=AL.mult, op1=op1), fd)

            def tt(out, in0, in1, op, fd, engines=V):
                ew(engines, lambda E: E.tensor_tensor(
                    out=out, in0=in0, in1=in1, op=op), fd)

            def acopy(out, in_, fd):
                e = led.pick(("a", "v"), fd)
                E = led.eng(e)
                if e == "a":
                    E.copy(out=out, in_=in_)
                else:
                    E.tensor_copy(out=out, in_=in_)

            cA, sA = 9, 19

            def ctf(q, bcast=None):
                return xaf(cA + q, 1) if bcast is None else xaf(cA + q, bcast=bcast)

            def stf(q, bcast=None):
                return xaf(sA + q, 1) if bcast is None else xaf(sA + q, bcast=bcast)

            # ---------------- stage A: alpha normalization ------------------
            tt(tb1(0, 10), fap(xa, cA * W, [[1, 10 * W]]),
               fap(xa, cA * W, [[1, 10 * W]]), AL.mult, 10 * W, engines=VG)
            tt(tb1(10, 10), fap(xa, sA * W, [[1, 10 * W]]),
               fap(xa, sA * W, [[1, 10 * W]]), AL.mult, 10 * W, engines=VG)
            tt(tb1(0, 10), tb1(0, 10), tb1(10, 10), AL.add, 10 * W, engines=VG)
            nc.scalar.sqrt(out=tb1(10, 10), in_=tb1(0, 10))
            ew(VG, lambda E: E.tensor_scalar(
                out=tb1(10, 10), in0=tb1(10, 10), scalar1=1e-6, scalar2=None,
                op0=AL.add), 10 * W)
            nc.vector.reciprocal(out=tb1(0, 10), in_=tb1(10, 10))
            tt(fap(xa, cA * W, [[1, 10 * W]]), fap(xa, cA * W, [[1, 10 * W]]),
               tb1(0, 10), AL.mult, 10 * W, engines=VG)
            tt(fap(xa, sA * W, [[1, 10 * W]]), fap(xa, sA * W, [[1, 10 * W]]),
               tb1(0, 10), AL.mult, 10 * W, engines=VG)

            # ---------------- stage B: frame 0 ------------------------------
            tt(tb1(0, 3), fap(xa, 6 * W, [[1, 3 * W]]),
               fap(xa, 3 * W, [[1, 3 * W]]), AL.subtract, 3 * W, engines=VG)
            tt(tb1(3, 3), fap(xa, 0, [[1, 3 * W]]),
               fap(xa, 3 * W, [[1, 3 * W]]), AL.subtract, 3 * W, engines=VG)
            tt(tb1(6, 3), tb1(0, 3), tb1(0, 3), AL.mult, 3 * W, engines=VG)
            nc.vector.tensor_reduce(
                out=tbf(9, 1), in_=fap(tb, 6 * W, [[1, W], [W, 3]]),
                axis=mybir.AxisListType.X, op=AL.add)
            nc.scalar.sqrt(out=tbf(10, 1), in_=tbf(9, 1))
            ew(VG, lambda E: E.tensor_scalar(
                out=tbf(10, 1), in0=tbf(10, 1), scalar1=1e-8, scalar2=None,
                op0=AL.add), W)
            nc.vector.reciprocal(out=tbf(9, 1), in_=tbf(10, 1))
            tt(rf(0, 0, 0, ni=3), tbf(0, 3), tbb(9, 3), AL.mult, 3 * W)
            tt(tbf(6, 3), rf(0, 0, 0, ni=3), tbf(3, 3), AL.mult, 3 * W)
            nc.vector.tensor_reduce(
                out=tbf(9, 1), in_=fap(tb, 6 * W, [[1, W], [W, 3]]),
                axis=mybir.AxisListType.X, op=AL.add)
            tt(tbf(6, 3), rf(0, 0, 0, ni=3), tbb(9, 3), AL.mult, 3 * W)
            tt(tb1(0, 3), tb1(3, 3), tb1(6, 3), AL.subtract, 3 * W, engines=VG)
            tt(tb1(6, 3), tb1(0, 3), tb1(0, 3), AL.mult, 3 * W, engines=VG)
            nc.vector.tensor_reduce(
                out=tbf(9, 1), in_=fap(tb, 6 * W, [[1, W], [W, 3]]),
                axis=mybir.AxisListType.X, op=AL.add)
            nc.scalar.sqrt(out=tbf(10, 1), in_=tbf(9, 1))
            ew(VG, lambda E: E.tensor_scalar(
                out=tbf(10, 1), in0=tbf(10, 1), scalar1=1e-8, scalar2=None,
                op0=AL.add), W)
            nc.vector.reciprocal(out=tbf(9, 1), in_=tbf(10, 1))
            tt(rf(0, 0, 1, ni=3), tbf(0, 3), tbb(9, 3), AL.mult, 3 * W)
            for cc in range(3):
                i1, i2 = (cc + 1) % 3, (cc + 2) % 3
                tt(tbf(6, 1), rf(0, i1, 0), rf(0, i2, 1), AL.mult, W)
                tt(tbf(7, 1), rf(0, i2, 0), rf(0, i1, 1), AL.mult, W)
                tt(rf(0, cc, 2), tbf(6, 1), tbf(7, 1), AL.subtract, W)
            acopy(rf(0, 0, 3, ni=3), xaf(3, 3), 3 * W)
            nc.gpsimd.memset(rff(108, 3), 0.0)
            nc.gpsimd.memset(rff(111, 1), 1.0)
            nc.gpsimd.memset(rff(140, 3), 0.0)
            nc.gpsimd.memset(rff(143, 1), 1.0)

            # ---------------- stage C: rot-axis matrices --------------------
            for (q, ba, bb, bc, t0) in ((7, PT_A1, PT_B1, PT_C1, 0),
                                        (8, PT_A2, PT_B2, PT_C2, 9)):
                tt(taf(t0, 9), ctf(q, bcast=9), ptb(ba, 9), AL.mult, 9 * W)
                tt(tbf(12, 9), stf(q, bcast=9), ptb(bb, 9), AL.mult, 9 * W)
                tt(taf(t0, 9), taf(t0, 9), tbf(12, 9), AL.add, 9 * W)
                tt(taf(t0, 9), taf(t0, 9), ptb(bc, 9), AL.add, 9 * W)

            # ---------------- stage D: frame 8 rows0-2 ----------------------
            def p1f(j, ni=3):           # P1 col j: feats 21+j*3 .. +3
                return tbf(21 + j * 3, ni)

            for (src_t0, dst) in ((0, "p1"), (9, "rtfs8")):
                for j in range(3):
                    for k in range(3):
                        in0 = rf(0, 0, k, ni=3) if dst == "p1" else p1f(k)
                        outp = p1f(j) if dst == "p1" else rf(8, 0, j, ni=3)
                        a1 = fap(ta, (src_t0 + k * 3 + j) * W, [[0, 3], [1, W]])
                        if k == 0:
                            tt(outp, in0, a1, AL.mult, 3 * W)
                        else:
                            tt(tbf(30, 3), in0, a1, AL.mult, 3 * W)
                            tt(outp, outp, tbf(30, 3), AL.add, 3 * W)
            acopy(rf(8, 0, 3, ni=3), rf(0, 0, 3, ni=3), 3 * W)

            # ------------- chain steps rows0-2 ------------------------------
            def mat_cs(q, ct_at, st_at):
                """materialize ct/st replicated x3 into contiguous tb."""
                acopy(tbf(ct_at, 3), ctf(q, bcast=3), 3 * W)
                acopy(tbf(st_at, 3), stf(q, bcast=3), 3 * W)

            def chain_rows(frame, left, kk, q, g0, q0, ct_at, st_at):
                def gcol(j, one_d=False):   # contiguous 3-feat block
                    f0 = g0 + (j - 1) * 3
                    return tb1(f0, 3) if one_d else tbf(f0, 3)

                mat_cs(q, ct_at, st_at)
                for j in range(4):
                    outp = rf(frame, 0, j, ni=3) if j in (0, 3) else gcol(j)
                    for m in range(4):
                        c = PT_RT + kk * 16 + m * 4 + j
                        if m == 0:
                            ts_mul(outp, rf(left, 0, 0, ni=3), pts(c), 3 * W)
                        else:
                            stt(outp, rf(left, 0, m, ni=3), c, 3 * W)
                # rotX cols 1,2 (muls on contiguous views, GPS-eligible)
                tt(tb1(q0, 3), gcol(1, True), tb1(ct_at, 3), AL.mult, 3 * W,
                   engines=VG)
                tt(tb1(q0 + 3, 3), gcol(2, True), tb1(st_at, 3), AL.mult,
                   3 * W, engines=VG)
                tt(rf(frame, 0, 1, ni=3), tbf(q0, 3), tbf(q0 + 3, 3),
                   AL.add, 3 * W)
                tt(tb1(q0, 3), gcol(2, True), tb1(ct_at, 3), AL.mult, 3 * W,
                   engines=VG)
                tt(tb1(q0 + 3, 3), gcol(1, True), tb1(st_at, 3), AL.mult,
                   3 * W, engines=VG)
                tt(rf(frame, 0, 2, ni=3), tbf(q0, 3), tbf(q0 + 3, 3),
                   AL.subtract, 3 * W)

            def chain_row3_const(frame, kk, q, tmp):
                c3 = lambda j: PT_RT + kk * 16 + 12 + j
                const_write(r3f(frame, 0), c3(0), W)
                const_write(r3f(frame, 3), c3(3), W)
                ts_mul(r3f(frame, 1), ctf(q), pts(c3(1)), W)
                stt(r3f(frame, 1), stf(q), c3(2), W)
                ts_mul(r3f(frame, 2), ctf(q), pts(c3(2)), W)
                ts_mul(tbf(tmp, 1), stf(q), pts(c3(1)), W)
                tt(r3f(frame, 2), r3f(frame, 2), tbf(tmp, 1), AL.subtract, W,
                   engines=VG)

            def chain_row3_full(frame, left, kk, q, g0, tmp):
                c = lambda m, j: PT_RT + kk * 16 + m * 4 + j
                stage = {1: tbf(g0, 1), 2: tbf(g0 + 1, 1)}
                for j in range(4):
                    outp = r3f(frame, j) if j in (0, 3) else stage[j]
                    ts_mul(outp, r3f(left, 0), pts(c(0, j)), W)
                    for m in range(1, 4):
                        stt(outp, r3f(left, m), c(m, j), W)
                tt(r3f(frame, 1), stage[1], ctf(q), AL.mult, W, engines=VG)
                tt(tbf(tmp, 1), stage[2], stf(q), AL.mult, W, engines=VG)
                tt(r3f(frame, 1), r3f(frame, 1), tbf(tmp, 1), AL.add, W,
                   engines=VG)
                tt(r3f(frame, 2), stage[2], ctf(q), AL.mult, W, engines=VG)
                tt(tbf(tmp, 1), stage[1], stf(q), AL.mult, W, engines=VG)
                tt(r3f(frame, 2), r3f(frame, 2), tbf(tmp, 1), AL.subtract, W,
                   engines=VG)

            chain_rows(1, 0, 0, 0, g0=33, q0=39, ct_at=45, st_at=48)
            chain_row3_const(1, 0, 0, tmp=51)
            chain_rows(2, 0, 1, 1, g0=52, q0=58, ct_at=64, st_at=67)
            chain_row3_const(2, 1, 1, tmp=70)
            chain_rows(3, 0, 2, 2, g0=0, q0=6, ct_at=12, st_at=15)
            chain_row3_const(3, 2, 2, tmp=18)

            # ---------------- frame 4 rows0-2 -------------------------------
            def g3col(j, one_d=False):      # G3 cols contiguous at 21+3j
                return tb1(21 + j * 3, 3) if one_d else tbf(21 + j * 3, 3)

            mat_cs(3, 71, 74)               # ct3/st3 triples
            mat_cs(9, 45, 48)               # ct9/st9 triples (reuse E1)
            for j in range(4):
                outp = rf(4, 0, j, ni=3) if j == 3 else g3col(j)
                for m in range(4):
                    c = PT_RT + 3 * 16 + m * 4 + j
                    if m == 0:
                        ts_mul(outp, rf(8, 0, 0, ni=3), pts(c), 3 * W)
                    else:
                        stt(outp, rf(8, 0, m, ni=3), c, 3 * W)
            # H1 = g1*ct3 + g2*st3 (tb 30-33); col2 = g2*ct3 - g1*st3
            tt(tb1(30, 3), g3col(1, True), tb1(71, 3), AL.mult, 3 * W,
               engines=VG)
            tt(tb1(39, 3), g3col(2, True), tb1(74, 3), AL.mult, 3 * W,
               engines=VG)
            tt(tb1(30, 3), tb1(30, 3), tb1(39, 3), AL.add, 3 * W, engines=VG)
            tt(tb1(39, 3), g3col(2, True), tb1(71, 3), AL.mult, 3 * W,
               engines=VG)
            tt(tb1(42, 3), g3col(1, True), tb1(74, 3), AL.mult, 3 * W,
               engines=VG)
            tt(rf(4, 0, 2, ni=3), tbf(39, 3), tbf(42, 3), AL.subtract, 3 * W)
            # rotZ9: col0 = g0*c9 + H1*s9 ; col1 = H1*c9 - g0*s9
            tt(tb1(39, 3), g3col(0, True), tb1(45, 3), AL.mult, 3 * W,
               engines=VG)
            tt(tb1(42, 3), tb1(30, 3), tb1(48, 3), AL.mult, 3 * W, engines=VG)
            tt(rf(4, 0, 0, ni=3), tbf(39, 3), tbf(42, 3), AL.add, 3 * W)
            tt(tb1(39, 3), tb1(30, 3), tb1(45, 3), AL.mult, 3 * W, engines=VG)
            tt(tb1(42, 3), g3col(0, True), tb1(48, 3), AL.mult, 3 * W,
               engines=VG)
            tt(rf(4, 0, 1, ni=3), tbf(39, 3), tbf(42, 3), AL.subtract, 3 * W)
            # frame 4 row3: rt3row3 @ rotX3 @ rotZ9 (rt3row3 const)
            c3 = lambda j: PT_RT + 3 * 16 + 12 + j
            ts_mul(tbf(51, 1), ctf(3), pts(c3(1)), W)       # a1
            stt(tbf(51, 1), stf(3), c3(2), W)
            ts_mul(r3f(4, 2), ctf(3), pts(c3(2)), W)        # a2 direct
            ts_mul(tbf(19, 1), stf(3), pts(c3(1)), W)
            tt(r3f(4, 2), r3f(4, 2), tbf(19, 1), AL.subtract, W, engines=VG)
            const_write(r3f(4, 3), c3(3), W)
            ts_mul(r3f(4, 0), ctf(9), pts(c3(0)), W)        # a0*c9
            tt(tbf(19, 1), tbf(51, 1), stf(9), AL.mult, W, engines=VG)
            tt(r3f(4, 0), r3f(4, 0), tbf(19, 1), AL.add, W, engines=VG)
            tt(r3f(4, 1), tbf(51, 1), ctf(9), AL.mult, W, engines=VG)
            ts_mul(tbf(19, 1), stf(9), pts(c3(0)), W)       # a0*s9
            tt(r3f(4, 1), r3f(4, 1), tbf(19, 1), AL.subtract, W, engines=VG)

            for (fr_i, left, kk, q) in ((5, 4, 4, 4), (6, 5, 5, 5), (7, 6, 6, 6)):
                chain_rows(fr_i, left, kk, q, g0=21, q0=27, ct_at=33, st_at=36)
                chain_row3_full(fr_i, left, kk, q, g0=52, tmp=54)

            # ---------------- frame output DMA (contiguous) -----------------
            nc.sync.dma_start(
                out=d_rtf, in_=fap(rtfs, 0, [[1, 144 * W]], parts=P))

            # ---------------- einsum ----------------------------------------
            packed_off = 0
            col = 0
            while col < W:
                wch = min(Wc, W - col)
                ngrp = (wch + 3) // 4
                for g in range(ngrp):
                    pst = ps_tr.tile([128, 512], F32, tag="pstr")
                    nw = min(4, wch - g * 4)
                    for r in range(nw):
                        w = col + g * 4 + r
                        nc.tensor.matmul(
                            out=fap(pst[:], r * 128, [[1, 128]], parts=108),
                            lhsT=fap(rtfs, w, [[W, 108]]),
                            rhs=idn, is_transpose=True,
                            start=True, stop=True)
                    # scatter-copy psum -> TR (f32r rounding producer)
                    e = led.pick(("v", "a"), nw * 128)
                    cp_out = fap(tb, TR0 + g * 4, [[1, nw], [wch, 128]],
                                 parts=108)
                    cp_in = fap(pst[:], 0, [[128, nw], [1, 128]], parts=108)
                    if e == "a":
                        led.eng(e).copy(out=cp_out, in_=cp_in)
                    else:
                        led.eng(e).tensor_copy(out=cp_out, in_=cp_in)
                for s in range(NAA):
                    ns = int(n_s[s]); p0 = int(p_start[s])
                    if ns == 0:
                        continue
                    N = ns * wch
                    psm = ps_mm.tile([128, 512], F32, tag="psmm")
                    nc.tensor.matmul(
                        out=fap(psm[:], 0, [[1, N]], parts=81),
                        lhsT=fap(lt, s * 81, [[1, 81]], parts=108),
                        rhs=fap(tb, TR0 + p0 * wch, [[1, N]],
                                parts=108),
                        start=True, stop=True)
                    xo = xo_pool.tile([128, 512], F32, tag="xo")
                    e = led.pick(("v", "a"), N)
                    xo_out = fap(xo[:], 0, [[1, N]], parts=81)
                    xo_in = fap(psm[:], 0, [[1, N]], parts=81)
                    if e == "a":
                        led.eng(e).copy(out=xo_out, in_=xo_in)
                    else:
                        led.eng(e).tensor_copy(out=xo_out, in_=xo_in)
                    nc.sync.dma_start(
                        out=bass.AP(d_x3.tensor, packed_off,
                                    [[P * W, 81], [1, N]]),
                        in_=fap(xo[:], 0, [[1, N]], parts=81))
                    packed_off += N
                col += wch

    nc.compile()
    return nc


# ----------------------------------------------------------------------------
# kernel entry
# ----------------------------------------------------------------------------

_CACHE = {}


def kernel(seq, xyz, alphas, base_indices, RTs_in_base_frame,
           xyzs_in_base_frame, allatom_mask):
    seq = np.asarray(seq).astype(np.int64)
    xyz = np.asarray(xyz, np.float32)
    alphas = np.asarray(alphas, np.float32)

    key = hashlib.sha1(
        seq.tobytes() + np.asarray(base_indices).astype(np.int64).tobytes()
        + np.asarray(RTs_in_base_frame, np.float32).tobytes()
        + np.asarray(xyzs_in_base_frame, np.float32).tobytes()
        + np.asarray(allatom_mask).astype(np.uint8).tobytes()).hexdigest()

    if key in _CACHE:
        plan, tables, nc, tokmaps, valids = _CACHE[key]
    else:
        tables = make_tables(base_indices, RTs_in_base_frame,
                             xyzs_in_base_frame, allatom_mask)
        plan = make_plan(seq)
        tokmaps, valids = [], []
        for c in range(N_CORES):
            tm, vd = core_slots(plan, seq[c * BPC:(c + 1) * BPC].reshape(-1))
            tokmaps.append(tm); valids.append(vd)
        nc = build_program(plan)
        _CACHE[key] = (plan, tables, nc, tokmaps, valids)

    ptab = build_ptab(plan, tables)
    ltp = np.ascontiguousarray(
        tables["LT"].transpose(1, 0, 2).reshape(108, NAA * 81))
    ident = np.eye(128, dtype=np.float32)

    in_maps = []
    for c in range(N_CORES):
        xa = pack_inputs(plan, tokmaps[c],
                         xyz[c * BPC:(c + 1) * BPC],
                         alphas[c * BPC:(c + 1) * BPC])
        in_maps.append({"xa": xa, "ptab": ptab, "lt": ltp, "ident": ident})

    from concourse.bass_utils import run_bass_kernel_spmd
    global _LAST
    _LAST = (nc, in_maps)
    res = run_bass_kernel_spmd(nc, in_maps, core_ids=list(range(N_CORES)))

    rtf_cores = [res.results[c]["rtf"] for c in range(N_CORES)]
    xyz3s_cores = [res.results[c]["xyz3s"] for c in range(N_CORES)]
    return assemble_outputs(plan, tables, seq, rtf_cores, xyz3s_cores,
                            tokmaps, valids)


_LAST = None


def _ensure_ntff_hook():
    """Shim antenv.axon_hooks if the image lacks it (boot degrades silently)."""
    import types
    try:
        from antenv.axon_hooks import get_axon_ntff_profile_hook  # noqa: F401
        return
    except ImportError:
        pass
    import antenv
    from trn_agent_boot.trn_boot import _ntff_profile_via_ctypes
    hook = [_ntff_profile_via_ctypes("/opt/axon/libaxon_pjrt.so")]
    mod = types.ModuleType("antenv.axon_hooks")
    mod.get_axon_ntff_profile_hook = lambda: hook[0]
    mod.set_axon_ntff_profile_hook = lambda h: hook.__setitem__(0, h)
    sys.modules["antenv.axon_hooks"] = mod
    antenv.axon_hooks = mod


def bench_hw(trace=True):
    """Re-run the last-compiled program with NTFF tracing for HW timing."""
    if _LAST is None:
        return None
    if trace:
        _ensure_ntff_hook()
    from concourse.bass_utils import run_bass_kernel_spmd
    nc, in_maps = _LAST
    return run_bass_kernel_spmd(nc, in_maps, core_ids=list(range(N_CORES)),
                                trace=trace)
